# revision 1
# baseline (speedup 1.0000x reference)
import numpy as np

# CNN-biLSTM-CRF forward NLL, data-parallel over batch across 8 NeuronCores.
# Device computes the dominant batched matmul (biLSTM input projections for
# both directions, fused into one [1024,384]x[384,2048] matmul per core);
# host handles embedding gathers, the tiny char-CNN, the sequential LSTM
# recurrence and the CRF scan in fp32 numpy.

B, S, LW = 64, 128, 20
CHAR_E, CHAR_C = 30, 30
WORD_E = 300
H, NCLS = 256, 25
F = WORD_E + CHAR_C  # 330
KPAD = 384  # F padded to 3*128 for K-tiling
NCORES = 8
BC = B // NCORES  # 8 examples per core
R = BC * S  # 1024 rows per core
NW = 8 * H  # 2048 = both directions' 4H gates


def _build_nc():
    import concourse.bacc as bacc
    import concourse.mybir as mybir
    from concourse import tile

    nc = bacc.Bacc("TRN2", target_bir_lowering=False, debug=False,
                   num_devices=NCORES)
    featT = nc.dram_tensor("featT", [KPAD, R], mybir.dt.float32,
                           kind="ExternalInput")
    wT = nc.dram_tensor("wT", [KPAD, NW], mybir.dt.float32,
                        kind="ExternalInput")
    gx = nc.dram_tensor("gx", [R, NW], mybir.dt.float32,
                        kind="ExternalOutput")
    f32 = mybir.dt.float32
    with tile.TileContext(nc) as tc:
        with tc.tile_pool(name="lhs", bufs=1) as lp, \
             tc.tile_pool(name="rhs", bufs=1) as rp, \
             tc.tile_pool(name="ob", bufs=4) as op_, \
             tc.tile_pool(name="ps", bufs=4, space="PSUM") as pp:
            lhs, rhs = [], []
            for k in range(3):
                lt = lp.tile([128, R], f32, tag=f"l{k}")
                nc.sync.dma_start(lt[:, :], featT[k * 128:(k + 1) * 128, :])
                lhs.append(lt)
                rt = rp.tile([128, NW], f32, tag=f"r{k}")
                nc.sync.dma_start(rt[:, :], wT[k * 128:(k + 1) * 128, :])
                rhs.append(rt)
            for m in range(R // 128):
                for n in range(NW // 512):
                    ps = pp.tile([128, 512], f32)
                    for k in range(3):
                        nc.tensor.matmul(
                            ps[:, :],
                            lhs[k][:, m * 128:(m + 1) * 128],
                            rhs[k][:, n * 512:(n + 1) * 512],
                            start=(k == 0), stop=(k == 2))
                    ot = op_.tile([128, 512], f32)
                    nc.vector.tensor_copy(ot[:, :], ps[:, :])
                    nc.sync.dma_start(
                        gx[m * 128:(m + 1) * 128, n * 512:(n + 1) * 512],
                        ot[:, :])
    nc.compile()
    return nc


_NC_CACHE = {}


LAST_DEVICE_NS = [0]


def _run_device(featT_shards, wTp):
    import time
    from concourse.bass_utils import run_bass_kernel_spmd
    if "nc" not in _NC_CACHE:
        _NC_CACHE["nc"] = _build_nc()
    nc = _NC_CACHE["nc"]
    in_maps = [{"featT": featT_shards[c], "wT": wTp} for c in range(NCORES)]
    t0 = time.time()
    res = run_bass_kernel_spmd(nc, in_maps, core_ids=list(range(NCORES)))
    LAST_DEVICE_NS[0] = int((time.time() - t0) * 1e9)
    return [r["gx"] for r in res.results]


def _sigmoid(x):
    return 1.0 / (1.0 + np.exp(-x))


def _logsumexp(x, axis):
    m = np.max(x, axis=axis, keepdims=True)
    return (m + np.log(np.sum(np.exp(x - m), axis=axis,
                              keepdims=True))).squeeze(axis)


def kernel(word_table, char_table, conv_w, conv_b, w_ih_f, w_hh_f, b_f,
           w_ih_r, w_hh_r, b_r, lin_w, lin_b, start_t, end_t, trans,
           sent, word, tag, mask):
    word_table = np.asarray(word_table, np.float32)
    char_table = np.asarray(char_table, np.float32)
    conv_w = np.asarray(conv_w, np.float32)
    conv_b = np.asarray(conv_b, np.float32)
    lin_w = np.asarray(lin_w, np.float32)
    lin_b = np.asarray(lin_b, np.float32)
    start_t = np.asarray(start_t, np.float32)
    end_t = np.asarray(end_t, np.float32)
    trans = np.asarray(trans, np.float32)
    sent_i = np.asarray(sent).astype(np.int64)
    word_i = np.asarray(word).astype(np.int64)
    tag_i = np.asarray(tag).astype(np.int64)
    mask_b = np.asarray(mask).astype(bool)

    # --- char CNN (host: tiny) ---
    ct = char_table.copy()
    ct[0] = 0.0
    cemb = ct[word_i.reshape(-1)].reshape(B * S, LW, CHAR_E)
    pad = np.zeros((B * S, LW + 2, CHAR_E), np.float32)
    pad[:, 1:LW + 1, :] = cemb
    conv = np.zeros((B * S, LW, CHAR_C), np.float32)
    for dk in range(3):
        conv += pad[:, dk:dk + LW, :] @ conv_w[:, :, dk].T
    conv += conv_b[None, None, :]
    char_feat = conv.max(axis=1).reshape(B, S, CHAR_C)

    # --- word embedding + concat ---
    wemb = word_table[sent_i.reshape(-1)].reshape(B, S, WORD_E)
    feat = np.concatenate([wemb, char_feat], axis=2)  # [B,S,F]

    # --- device: input projections for both LSTM directions ---
    wcat = np.concatenate([w_ih_f, w_ih_r], axis=0).astype(np.float32)  # [2048,330]
    wTp = np.zeros((KPAD, NW), np.float32)
    wTp[:F] = np.ascontiguousarray(wcat.T)
    shards = []
    for c in range(NCORES):
        fc = feat[c * BC:(c + 1) * BC].reshape(R, F)  # [1024,330]
        fT = np.zeros((KPAD, R), np.float32)
        fT[:F] = np.ascontiguousarray(fc.T)
        shards.append(fT)
    gx_shards = _run_device(shards, wTp)
    gx = np.concatenate(
        [g.reshape(BC, S, NW) for g in gx_shards], axis=0)  # [B,S,2048]
    gx_f = gx[:, :, :4 * H] + np.asarray(b_f, np.float32)[None, None, :]
    gx_r = gx[:, :, 4 * H:] + np.asarray(b_r, np.float32)[None, None, :]

    # --- LSTM recurrences (host) ---
    def run_dir(gxd, w_hh, reverse):
        w_hh_t = np.ascontiguousarray(np.asarray(w_hh, np.float32).T)
        h = np.zeros((B, H), np.float32)
        c = np.zeros((B, H), np.float32)
        hs = np.zeros((S, B, H), np.float32)
        order = range(S - 1, -1, -1) if reverse else range(S)
        for t in order:
            g = gxd[:, t] + h @ w_hh_t
            i = _sigmoid(g[:, :H])
            f = _sigmoid(g[:, H:2 * H])
            gg = np.tanh(g[:, 2 * H:3 * H])
            o = _sigmoid(g[:, 3 * H:])
            c = f * c + i * gg
            h = o * np.tanh(c)
            hs[t] = h
        return hs

    hf = run_dir(gx_f, w_hh_f, False)
    hr = run_dir(gx_r, w_hh_r, True)
    hcat = np.concatenate([hf, hr], axis=-1)  # [S,B,2H]
    em = hcat @ lin_w.T + lin_b  # [S,B,NCLS]

    # --- CRF NLL (host) ---
    tg = tag_i.T  # [S,B]
    mk = mask_b.T.astype(np.float32)
    bidx = np.arange(B)
    em_tag = np.take_along_axis(em, tg[..., None], axis=-1)[..., 0]
    tr = trans[tg[:-1], tg[1:]]
    score = start_t[tg[0]] + em_tag[0] + np.sum(
        mk[1:] * (tr + em_tag[1:]), axis=0)
    last = mk.sum(0).astype(np.int64) - 1
    score = score + end_t[tg[last, bidx]]
    alpha = start_t[None, :] + em[0]
    for t in range(1, S):
        nxt = _logsumexp(
            alpha[:, :, None] + trans[None, :, :] + em[t][:, None, :], axis=1)
        alpha = np.where(mk[t][:, None] > 0, nxt, alpha)
    logZ = _logsumexp(alpha + end_t[None, :], axis=1)
    return np.asarray(-np.sum(score - logZ), np.float32)



# revision 8
# speedup vs baseline: 2.4046x; 2.4046x over previous
import numpy as np

# CNN-biLSTM-CRF forward NLL, data-parallel over batch across 8 NeuronCores.
# Device computes the input projections, the full biLSTM recurrence and the
# emission linear layer; host handles embedding gathers, the tiny char-CNN
# and the CRF scan. Only the [1024,25] emissions per core come back, so the
# axon transfer (the dominant cost) is ~26x smaller than shipping gates.

B, S, LW = 64, 128, 20
CHAR_E, CHAR_C = 30, 30
WORD_E = 300
H, NCLS = 256, 25
F = WORD_E + CHAR_C  # 330
KA = F + 1  # ones row folds the gate bias into the matmul
NCORES = 8
BC = B // NCORES  # 8 examples per core
R = BC * S  # 1024 rows per core, time-major: row = t*BC + e
NW = 8 * H  # 2048 = both directions' 4H gates
H4 = 4 * H

# gate order on device: i, f, o, g  (sigmoid on [0:3H), tanh on [3H:4H))
_GATE_PERM = np.concatenate([
    np.arange(0, H),            # i
    np.arange(H, 2 * H),        # f
    np.arange(3 * H, 4 * H),    # o
    np.arange(2 * H, 3 * H),    # g
])


def _build_nc():
    import concourse.bacc as bacc
    import concourse.mybir as mybir
    from concourse import tile

    f32 = mybir.dt.float32
    bf16 = mybir.dt.bfloat16
    AF = mybir.ActivationFunctionType

    nc = bacc.Bacc("TRN2", target_bir_lowering=False, debug=False,
                   num_devices=NCORES)
    featT = nc.dram_tensor("featT", [KA, R], bf16, kind="ExternalInput")
    wT = nc.dram_tensor("wT", [KA, NW], bf16, kind="ExternalInput")
    whhT_f = nc.dram_tensor("whhT_f", [H, H4], bf16, kind="ExternalInput")
    whhT_r = nc.dram_tensor("whhT_r", [H, H4], bf16, kind="ExternalInput")
    linT = nc.dram_tensor("linT", [2 * H, NCLS], bf16, kind="ExternalInput")
    em = nc.dram_tensor("em", [R, NCLS], f32, kind="ExternalOutput")

    ksz = [128, 128, KA - 256]
    ko = [0, 128, 256]

    with tile.TileContext(nc) as tc:
        with tc.tile_pool(name="wp", bufs=1) as wp, \
             tc.tile_pool(name="gxp", bufs=1) as gxp, \
             tc.tile_pool(name="stp", bufs=1) as stp, \
             tc.tile_pool(name="wk", bufs=3) as wk, \
             tc.tile_pool(name="pm", bufs=4, space="PSUM") as pm, \
             tc.tile_pool(name="pg", bufs=4, space="PSUM") as pg:

            # moving featT K-tiles and stationary wT K-tiles
            ft, wt = [], []
            for k in range(3):
                t1 = wp.tile([ksz[k], R], bf16, name=f"ft{k}", tag=f"ft{k}")
                nc.sync.dma_start(t1[:, :], featT[ko[k]:ko[k] + ksz[k], :])
                ft.append(t1)
                t2 = wp.tile([ksz[k], NW], bf16, name=f"wt{k}", tag=f"wt{k}")
                nc.sync.dma_start(t2[:, :], wT[ko[k]:ko[k] + ksz[k], :])
                wt.append(t2)
            whh = {}
            for d, dram in ((0, whhT_f), (1, whhT_r)):
                t0 = wp.tile([128, H4], bf16, name=f"whh{d}0", tag=f"whh{d}0")
                t1 = wp.tile([128, H4], bf16, name=f"whh{d}1", tag=f"whh{d}1")
                nc.sync.dma_start(t0[:, :], dram[0:128, :])
                nc.sync.dma_start(t1[:, :], dram[128:256, :])
                whh[d] = (t0, t1)
            lint = []
            for k in range(4):
                t = wp.tile([128, NCLS], bf16, name=f"lin{k}", tag=f"lin{k}")
                nc.sync.dma_start(t[:, :], linT[k * 128:(k + 1) * 128, :])
                lint.append(t)

            # gxT per direction: [128, S, 8, BC] fp32
            #   gxT_d[p, t, j, e] = gate (j*128+p) of dir d at time t, example e
            #   within-dir gate-slice order j: i0 i1 f0 f1 o0 o1 g0 g1
            gxT = [gxp.tile([128, S, 8, BC], f32, name=f"gxT{d}", tag=f"gxT{d}")
                   for d in range(2)]
            for d in range(2):
                for j in range(8):
                    gi = d * 8 + j
                    for rc in range(2):
                        ps = pm.tile([128, S // 2, BC], f32, tag="mm")
                        for k in range(3):
                            nc.tensor.matmul(
                                ps[:, :, :],
                                wt[k][:, gi * 128:(gi + 1) * 128],
                                ft[k][:, rc * 512:(rc + 1) * 512],
                                start=(k == 0), stop=(k == 2))
                        nc.scalar.copy(
                            gxT[d][:, rc * (S // 2):(rc + 1) * (S // 2), j, :],
                            ps[:, :, :])

            # hcat[0,1] = fwd h dims 0:128/128:256, hcat[2,3] = rev; col=t*8+e
            hcat = [stp.tile([128, R], bf16, name=f"hcat{j}", tag=f"hcat{j}")
                    for j in range(4)]
            # c state per dir: [128, 2, BC] (h dims 0:128 | 128:256)
            cst = [stp.tile([128, 2, BC], f32, name=f"c{d}", tag=f"c{d}")
                   for d in range(2)]

            for step in range(S):
                ts = [step, S - 1 - step]  # fwd time, rev time
                for d in range(2):
                    tt = ts[d]
                    c_sb = cst[d]
                    if step == 0:
                        gsl = gxT[d][:, tt]
                    else:
                        pt = tt - 1 if d == 0 else tt + 1
                        ps = pg.tile([128, 8, BC], f32, tag="rps")
                        w0, w1 = whh[d]
                        h0 = hcat[2 * d][:, pt * BC:(pt + 1) * BC]
                        h1 = hcat[2 * d + 1][:, pt * BC:(pt + 1) * BC]
                        for j in range(8):
                            nc.tensor.matmul(ps[:, j, :],
                                             w0[:, j * 128:(j + 1) * 128], h0,
                                             start=True, stop=False)
                            nc.tensor.matmul(ps[:, j, :],
                                             w1[:, j * 128:(j + 1) * 128], h1,
                                             start=False, stop=True)
                        g_sb = wk.tile([128, 8, BC], f32, tag="g")
                        nc.vector.tensor_add(g_sb[:, :, :], ps[:, :, :],
                                             gxT[d][:, tt])
                        gsl = g_sb
                    acts = wk.tile([128, 8, BC], f32, tag="acts")
                    nc.scalar.activation(acts[:, 0:6, :], gsl[:, 0:6, :],
                                         AF.Sigmoid)
                    nc.scalar.activation(acts[:, 6:8, :], gsl[:, 6:8, :],
                                         AF.Tanh)
                    if step == 0:
                        nc.vector.tensor_mul(c_sb[:, :, :], acts[:, 0:2, :],
                                             acts[:, 6:8, :])
                    else:
                        fc = wk.tile([128, 2, BC], f32, tag="fc")
                        nc.vector.tensor_mul(fc[:, :, :], acts[:, 2:4, :],
                                             c_sb[:, :, :])
                        nc.vector.tensor_mul(c_sb[:, :, :], acts[:, 0:2, :],
                                             acts[:, 6:8, :])
                        nc.vector.tensor_add(c_sb[:, :, :], c_sb[:, :, :],
                                             fc[:, :, :])
                    th = wk.tile([128, 2, BC], f32, tag="th")
                    nc.scalar.activation(th[:, :, :], c_sb[:, :, :], AF.Tanh)
                    nc.vector.tensor_mul(hcat[2 * d][:, tt * BC:(tt + 1) * BC],
                                         acts[:, 4, :], th[:, 0, :])
                    nc.vector.tensor_mul(
                        hcat[2 * d + 1][:, tt * BC:(tt + 1) * BC],
                        acts[:, 5, :], th[:, 1, :])

            for m in range(8):
                pe = pm.tile([128, NCLS], f32, tag="mm")
                for k in range(4):
                    nc.tensor.matmul(pe[:, :], hcat[k][:, m * 128:(m + 1) * 128],
                                     lint[k][:, :], start=(k == 0),
                                     stop=(k == 3))
                eo = wk.tile([128, NCLS], f32, tag="emo")
                nc.any.tensor_copy(eo[:, :], pe[:, :])
                nc.sync.dma_start(em[m * 128:(m + 1) * 128, :], eo[:, :])
    nc.compile()
    return nc


_NC_CACHE = {}
LAST_DEVICE_NS = [0]


def _run_device(in_maps):
    import time
    from concourse.bass_utils import run_bass_kernel_spmd
    if "nc" not in _NC_CACHE:
        _NC_CACHE["nc"] = _build_nc()
    nc = _NC_CACHE["nc"]
    t0 = time.time()
    res = run_bass_kernel_spmd(nc, in_maps, core_ids=list(range(NCORES)))
    LAST_DEVICE_NS[0] = int((time.time() - t0) * 1e9)
    return [r["em"] for r in res.results]


def _logsumexp(x, axis):
    m = np.max(x, axis=axis, keepdims=True)
    return (m + np.log(np.sum(np.exp(x - m), axis=axis,
                              keepdims=True))).squeeze(axis)


def kernel(word_table, char_table, conv_w, conv_b, w_ih_f, w_hh_f, b_f,
           w_ih_r, w_hh_r, b_r, lin_w, lin_b, start_t, end_t, trans,
           sent, word, tag, mask):
    import ml_dtypes
    bf = ml_dtypes.bfloat16
    word_table = np.asarray(word_table, np.float32)
    char_table = np.asarray(char_table, np.float32)
    conv_w = np.asarray(conv_w, np.float32)
    conv_b = np.asarray(conv_b, np.float32)
    lin_w = np.asarray(lin_w, np.float32)
    lin_b = np.asarray(lin_b, np.float32)
    start_t = np.asarray(start_t, np.float32)
    end_t = np.asarray(end_t, np.float32)
    trans = np.asarray(trans, np.float32)
    sent_i = np.asarray(sent).astype(np.int64)
    word_i = np.asarray(word).astype(np.int64)
    tag_i = np.asarray(tag).astype(np.int64)
    mask_b = np.asarray(mask).astype(bool)

    # --- char CNN (host: tiny) ---
    ct = char_table.copy()
    ct[0] = 0.0
    cemb = ct[word_i.reshape(-1)].reshape(B * S, LW, CHAR_E)
    pad = np.zeros((B * S, LW + 2, CHAR_E), np.float32)
    pad[:, 1:LW + 1, :] = cemb
    conv = np.zeros((B * S, LW, CHAR_C), np.float32)
    for dk in range(3):
        conv += pad[:, dk:dk + LW, :] @ conv_w[:, :, dk].T
    conv += conv_b[None, None, :]
    char_feat = conv.max(axis=1).reshape(B, S, CHAR_C)

    # --- word embedding + concat ---
    wemb = word_table[sent_i.reshape(-1)].reshape(B, S, WORD_E)
    feat = np.concatenate([wemb, char_feat], axis=2)  # [B,S,F]

    # --- device: projections + biLSTM + linear -> emissions ---
    gp = _GATE_PERM
    wcat = np.concatenate([np.asarray(w_ih_f, np.float32)[gp],
                           np.asarray(w_ih_r, np.float32)[gp]], axis=0)
    bcat = np.concatenate([np.asarray(b_f, np.float32)[gp],
                           np.asarray(b_r, np.float32)[gp]])
    wT = np.empty((KA, NW), np.float32)
    wT[:F] = wcat.T
    wT[F] = bcat
    wT = wT.astype(bf)
    whhT_f_a = np.ascontiguousarray(
        np.asarray(w_hh_f, np.float32)[gp].T).astype(bf)
    whhT_r_a = np.ascontiguousarray(
        np.asarray(w_hh_r, np.float32)[gp].T).astype(bf)
    linT = np.ascontiguousarray(lin_w.T).astype(bf)

    in_maps = []
    for c in range(NCORES):
        fc = feat[c * BC:(c + 1) * BC]  # [BC,S,F]
        fT = np.empty((KA, R), np.float32)
        fT[:F] = fc.transpose(2, 1, 0).reshape(F, R)
        fT[F] = 1.0
        in_maps.append(dict(featT=fT.astype(bf), wT=wT, whhT_f=whhT_f_a,
                            whhT_r=whhT_r_a, linT=linT))
    em_shards = _run_device(in_maps)
    em = np.empty((S, B, NCLS), np.float32)
    for c in range(NCORES):
        em[:, c * BC:(c + 1) * BC, :] = em_shards[c].reshape(S, BC, NCLS)
    em += lin_b[None, None, :]

    # --- CRF NLL (host) ---
    tg = tag_i.T  # [S,B]
    mk = mask_b.T.astype(np.float32)
    bidx = np.arange(B)
    em_tag = np.take_along_axis(em, tg[..., None], axis=-1)[..., 0]
    tr = trans[tg[:-1], tg[1:]]
    score = start_t[tg[0]] + em_tag[0] + np.sum(
        mk[1:] * (tr + em_tag[1:]), axis=0)
    last = mk.sum(0).astype(np.int64) - 1
    score = score + end_t[tg[last, bidx]]
    alpha = start_t[None, :] + em[0]
    for t in range(1, S):
        nxt = _logsumexp(
            alpha[:, :, None] + trans[None, :, :] + em[t][:, None, :], axis=1)
        alpha = np.where(mk[t][:, None] > 0, nxt, alpha)
    logZ = _logsumexp(alpha + end_t[None, :], axis=1)
    return np.asarray(-np.sum(score - logZ), np.float32)


# revision 10
# speedup vs baseline: 5.3530x; 2.2262x over previous
import numpy as np

# CNN-biLSTM-CRF forward NLL, data-parallel over batch across 8 NeuronCores.
# Device computes the input projections, the full biLSTM recurrence and the
# emission linear layer; host handles embedding gathers, the tiny char-CNN
# and the CRF scan. Only the [1024,25] emissions per core come back, so the
# axon transfer (the dominant cost) is ~26x smaller than shipping gates.

B, S, LW = 64, 128, 20
CHAR_E, CHAR_C = 30, 30
WORD_E = 300
H, NCLS = 256, 25
F = WORD_E + CHAR_C  # 330
KA = F + 1  # ones row folds the gate bias into the matmul
NCORES = 8
BC = B // NCORES  # 8 examples per core
R = BC * S  # 1024 rows per core, time-major: row = t*BC + e
NW = 8 * H  # 2048 = both directions' 4H gates
H4 = 4 * H

# gate order on device: i, f, o, g  (sigmoid on [0:3H), tanh on [3H:4H))
_GATE_PERM = np.concatenate([
    np.arange(0, H),            # i
    np.arange(H, 2 * H),        # f
    np.arange(3 * H, 4 * H),    # o
    np.arange(2 * H, 3 * H),    # g
])


def _build_nc():
    import concourse.bacc as bacc
    import concourse.mybir as mybir
    from concourse import tile

    f32 = mybir.dt.float32
    bf16 = mybir.dt.bfloat16
    AF = mybir.ActivationFunctionType

    nc = bacc.Bacc("TRN2", target_bir_lowering=False, debug=False,
                   num_devices=NCORES)
    featT = nc.dram_tensor("featT", [KA, R], bf16, kind="ExternalInput")
    wT = nc.dram_tensor("wT", [KA, NW], bf16, kind="ExternalInput")
    whhT_f = nc.dram_tensor("whhT_f", [H, H4], bf16, kind="ExternalInput")
    whhT_r = nc.dram_tensor("whhT_r", [H, H4], bf16, kind="ExternalInput")
    linT = nc.dram_tensor("linT", [2 * H, NCLS], bf16, kind="ExternalInput")
    em = nc.dram_tensor("em", [R, NCLS], f32, kind="ExternalOutput")

    ksz = [128, 128, KA - 256]
    ko = [0, 128, 256]

    with tile.TileContext(nc) as tc:
        with tc.tile_pool(name="wp", bufs=1) as wp, \
             tc.tile_pool(name="gxp", bufs=1) as gxp, \
             tc.tile_pool(name="stp", bufs=1) as stp, \
             tc.tile_pool(name="wk", bufs=3) as wk, \
             tc.tile_pool(name="pm", bufs=2, space="PSUM") as pm, \
             tc.tile_pool(name="pg", bufs=2, space="PSUM") as pg:

            # moving featT K-tiles and stationary wT K-tiles
            ft, wt = [], []
            for k in range(3):
                t1 = wp.tile([ksz[k], R], bf16, name=f"ft{k}", tag=f"ft{k}")
                nc.sync.dma_start(t1[:, :], featT[ko[k]:ko[k] + ksz[k], :])
                ft.append(t1)
                t2 = wp.tile([ksz[k], NW], bf16, name=f"wt{k}", tag=f"wt{k}")
                nc.sync.dma_start(t2[:, :], wT[ko[k]:ko[k] + ksz[k], :])
                wt.append(t2)
            whh = {}
            for d, dram in ((0, whhT_f), (1, whhT_r)):
                t0 = wp.tile([128, H4], bf16, name=f"whh{d}0", tag=f"whh{d}0")
                t1 = wp.tile([128, H4], bf16, name=f"whh{d}1", tag=f"whh{d}1")
                nc.sync.dma_start(t0[:, :], dram[0:128, :])
                nc.sync.dma_start(t1[:, :], dram[128:256, :])
                whh[d] = (t0, t1)
            lint = []
            for k in range(4):
                t = wp.tile([128, NCLS], bf16, name=f"lin{k}", tag=f"lin{k}")
                nc.sync.dma_start(t[:, :], linT[k * 128:(k + 1) * 128, :])
                lint.append(t)

            # gxT per direction: [128, S, 8, BC] fp32
            #   gxT_d[p, t, j, e] = gate (j*128+p) of dir d at time t, example e
            #   within-dir gate-slice order j: i0 i1 f0 f1 o0 o1 g0 g1
            gxT = [gxp.tile([128, S, 8, BC], f32, name=f"gxT{d}", tag=f"gxT{d}")
                   for d in range(2)]
            for d in range(2):
                for j in range(8):
                    gi = d * 8 + j
                    for rc in range(2):
                        ps = pm.tile([128, S // 2, BC], f32, tag="mm")
                        for k in range(3):
                            nc.tensor.matmul(
                                ps[:, :, :],
                                wt[k][:, gi * 128:(gi + 1) * 128],
                                ft[k][:, rc * 512:(rc + 1) * 512],
                                start=(k == 0), stop=(k == 2))
                        nc.scalar.copy(
                            gxT[d][:, rc * (S // 2):(rc + 1) * (S // 2), j, :],
                            ps[:, :, :])

            # hcat[0,1] = fwd h dims 0:128/128:256, hcat[2,3] = rev; col=t*8+e
            hcat = [stp.tile([128, R], bf16, name=f"hcat{j}", tag=f"hcat{j}")
                    for j in range(4)]
            # c state per dir: [128, 2, BC] (h dims 0:128 | 128:256)
            cst = [stp.tile([128, 2, BC], f32, name=f"c{d}", tag=f"c{d}")
                   for d in range(2)]

            from concourse.bass import ds

            def lstm_cell(d, gsl, first, hw_cols):
                """One LSTM cell update for direction d reading gates from
                gsl ([128,(1,)8,BC] pre-activation) and writing h to
                hcat[2d..2d+1][:, hw_cols]."""
                c_sb = cst[d]
                acts = wk.tile([128, 8, BC], f32, name=f"acts{d}",
                               tag=f"acts{d}")
                nc.scalar.activation(acts[:, 0:6, :], gsl[:, 0:6, :],
                                     AF.Sigmoid)
                nc.scalar.activation(acts[:, 6:8, :], gsl[:, 6:8, :], AF.Tanh)
                if first:
                    nc.vector.tensor_mul(c_sb[:, :, :], acts[:, 0:2, :],
                                         acts[:, 6:8, :])
                else:
                    fc = wk.tile([128, 2, BC], f32, name=f"fc{d}", tag=f"fc{d}")
                    nc.vector.tensor_mul(fc[:, :, :], acts[:, 2:4, :],
                                         c_sb[:, :, :])
                    nc.vector.tensor_mul(c_sb[:, :, :], acts[:, 0:2, :],
                                         acts[:, 6:8, :])
                    nc.vector.tensor_add(c_sb[:, :, :], c_sb[:, :, :],
                                         fc[:, :, :])
                th = wk.tile([128, 2, BC], f32, name=f"th{d}", tag=f"th{d}")
                nc.scalar.activation(th[:, :, :], c_sb[:, :, :], AF.Tanh)
                nc.vector.tensor_mul(hcat[2 * d][:, hw_cols],
                                     acts[:, 4, :], th[:, 0, :])
                nc.vector.tensor_mul(hcat[2 * d + 1][:, hw_cols],
                                     acts[:, 5, :], th[:, 1, :])

            # step 0 (no h feedback): gates come straight from gxT
            lstm_cell(0, gxT[0][:, 0], True, slice(0, BC))
            lstm_cell(1, gxT[1][:, S - 1], True, slice((S - 1) * BC, S * BC))

            # steps 1..S-1 as a hardware loop; sv = step index
            with tc.For_i(1, S, 1) as sv:
                for d in range(2):
                    # fwd: tt=sv prev=sv-1 ; rev: tt=S-1-sv prev=S-sv
                    if d == 0:
                        gcol = sv
                        pcol = sv * BC - BC
                        wcol = sv * BC
                    else:
                        gcol = S - 1 - sv
                        pcol = S * BC - sv * BC
                        wcol = (S - 1) * BC - sv * BC
                    ps = pg.tile([128, 8, BC], f32, name=f"rps{d}",
                                 tag=f"rps{d}")
                    w0, w1 = whh[d]
                    h0 = hcat[2 * d][:, ds(pcol, BC)]
                    h1 = hcat[2 * d + 1][:, ds(pcol, BC)]
                    for j in range(8):
                        nc.tensor.matmul(ps[:, j, :],
                                         w0[:, j * 128:(j + 1) * 128], h0,
                                         start=True, stop=False)
                        nc.tensor.matmul(ps[:, j, :],
                                         w1[:, j * 128:(j + 1) * 128], h1,
                                         start=False, stop=True)
                    g_sb = wk.tile([128, 8, BC], f32, name=f"g{d}",
                                   tag=f"g{d}")
                    nc.vector.tensor_add(g_sb[:, :, :], ps[:, :, :],
                                         gxT[d][:, ds(gcol, 1)])
                    lstm_cell(d, g_sb, False, ds(wcol, BC))

            for m in range(8):
                pe = pm.tile([128, NCLS], f32, tag="mm")
                for k in range(4):
                    nc.tensor.matmul(pe[:, :], hcat[k][:, m * 128:(m + 1) * 128],
                                     lint[k][:, :], start=(k == 0),
                                     stop=(k == 3))
                eo = wk.tile([128, NCLS], f32, tag="emo")
                nc.any.tensor_copy(eo[:, :], pe[:, :])
                nc.sync.dma_start(em[m * 128:(m + 1) * 128, :], eo[:, :])
    nc.compile()
    return nc


_NC_CACHE = {}
LAST_DEVICE_NS = [0]


def _run_device(in_maps):
    import time
    from concourse.bass_utils import run_bass_kernel_spmd
    if "nc" not in _NC_CACHE:
        _NC_CACHE["nc"] = _build_nc()
    nc = _NC_CACHE["nc"]
    t0 = time.time()
    res = run_bass_kernel_spmd(nc, in_maps, core_ids=list(range(NCORES)))
    LAST_DEVICE_NS[0] = int((time.time() - t0) * 1e9)
    return [r["em"] for r in res.results]


def _logsumexp(x, axis):
    m = np.max(x, axis=axis, keepdims=True)
    return (m + np.log(np.sum(np.exp(x - m), axis=axis,
                              keepdims=True))).squeeze(axis)


def kernel(word_table, char_table, conv_w, conv_b, w_ih_f, w_hh_f, b_f,
           w_ih_r, w_hh_r, b_r, lin_w, lin_b, start_t, end_t, trans,
           sent, word, tag, mask):
    import ml_dtypes
    bf = ml_dtypes.bfloat16
    word_table = np.asarray(word_table, np.float32)
    char_table = np.asarray(char_table, np.float32)
    conv_w = np.asarray(conv_w, np.float32)
    conv_b = np.asarray(conv_b, np.float32)
    lin_w = np.asarray(lin_w, np.float32)
    lin_b = np.asarray(lin_b, np.float32)
    start_t = np.asarray(start_t, np.float32)
    end_t = np.asarray(end_t, np.float32)
    trans = np.asarray(trans, np.float32)
    sent_i = np.asarray(sent).astype(np.int64)
    word_i = np.asarray(word).astype(np.int64)
    tag_i = np.asarray(tag).astype(np.int64)
    mask_b = np.asarray(mask).astype(bool)

    # --- char CNN (host: tiny) ---
    ct = char_table.copy()
    ct[0] = 0.0
    cemb = ct[word_i.reshape(-1)].reshape(B * S, LW, CHAR_E)
    pad = np.zeros((B * S, LW + 2, CHAR_E), np.float32)
    pad[:, 1:LW + 1, :] = cemb
    conv = np.zeros((B * S, LW, CHAR_C), np.float32)
    for dk in range(3):
        conv += pad[:, dk:dk + LW, :] @ conv_w[:, :, dk].T
    conv += conv_b[None, None, :]
    char_feat = conv.max(axis=1).reshape(B, S, CHAR_C)

    # --- word embedding + concat ---
    wemb = word_table[sent_i.reshape(-1)].reshape(B, S, WORD_E)
    feat = np.concatenate([wemb, char_feat], axis=2)  # [B,S,F]

    # --- device: projections + biLSTM + linear -> emissions ---
    gp = _GATE_PERM
    wcat = np.concatenate([np.asarray(w_ih_f, np.float32)[gp],
                           np.asarray(w_ih_r, np.float32)[gp]], axis=0)
    bcat = np.concatenate([np.asarray(b_f, np.float32)[gp],
                           np.asarray(b_r, np.float32)[gp]])
    wT = np.empty((KA, NW), np.float32)
    wT[:F] = wcat.T
    wT[F] = bcat
    wT = wT.astype(bf)
    whhT_f_a = np.ascontiguousarray(
        np.asarray(w_hh_f, np.float32)[gp].T).astype(bf)
    whhT_r_a = np.ascontiguousarray(
        np.asarray(w_hh_r, np.float32)[gp].T).astype(bf)
    linT = np.ascontiguousarray(lin_w.T).astype(bf)

    in_maps = []
    for c in range(NCORES):
        fc = feat[c * BC:(c + 1) * BC]  # [BC,S,F]
        fT = np.empty((KA, R), np.float32)
        fT[:F] = fc.transpose(2, 1, 0).reshape(F, R)
        fT[F] = 1.0
        in_maps.append(dict(featT=fT.astype(bf), wT=wT, whhT_f=whhT_f_a,
                            whhT_r=whhT_r_a, linT=linT))
    em_shards = _run_device(in_maps)
    em = np.empty((S, B, NCLS), np.float32)
    for c in range(NCORES):
        em[:, c * BC:(c + 1) * BC, :] = em_shards[c].reshape(S, BC, NCLS)
    em += lin_b[None, None, :]

    # --- CRF NLL (host) ---
    tg = tag_i.T  # [S,B]
    mk = mask_b.T.astype(np.float32)
    bidx = np.arange(B)
    em_tag = np.take_along_axis(em, tg[..., None], axis=-1)[..., 0]
    tr = trans[tg[:-1], tg[1:]]
    score = start_t[tg[0]] + em_tag[0] + np.sum(
        mk[1:] * (tr + em_tag[1:]), axis=0)
    last = mk.sum(0).astype(np.int64) - 1
    score = score + end_t[tg[last, bidx]]
    alpha = start_t[None, :] + em[0]
    for t in range(1, S):
        nxt = _logsumexp(
            alpha[:, :, None] + trans[None, :, :] + em[t][:, None, :], axis=1)
        alpha = np.where(mk[t][:, None] > 0, nxt, alpha)
    logZ = _logsumexp(alpha + end_t[None, :], axis=1)
    return np.asarray(-np.sum(score - logZ), np.float32)


# revision 13
# speedup vs baseline: 8.1742x; 1.5270x over previous
import numpy as np

# CNN-biLSTM-CRF forward NLL, data-parallel over batch across 8 NeuronCores.
# Device computes the input projections, the full biLSTM recurrence and the
# emission linear layer; host handles embedding gathers, the tiny char-CNN
# and the CRF scan. Only the [1024,25] emissions per core come back, so the
# axon transfer (the dominant cost) is ~26x smaller than shipping gates.

B, S, LW = 64, 128, 20
CHAR_E, CHAR_C = 30, 30
WORD_E = 300
H, NCLS = 256, 25
F = WORD_E + CHAR_C  # 330
KA = F + 1  # ones row folds the gate bias into the matmul
NCORES = 8
BC = B // NCORES  # 8 examples per core
R = BC * S  # 1024 rows per core, time-major: row = t*BC + e
NW = 8 * H  # 2048 = both directions' 4H gates
H4 = 4 * H

# gate order on device: i, f, o, g  (sigmoid on [0:3H), tanh on [3H:4H))
_GATE_PERM = np.concatenate([
    np.arange(0, H),            # i
    np.arange(H, 2 * H),        # f
    np.arange(3 * H, 4 * H),    # o
    np.arange(2 * H, 3 * H),    # g
])


def _build_nc():
    import concourse.bacc as bacc
    import concourse.mybir as mybir
    from concourse import tile

    f32 = mybir.dt.float32
    bf16 = mybir.dt.bfloat16
    AF = mybir.ActivationFunctionType

    nc = bacc.Bacc("TRN2", target_bir_lowering=False, debug=False,
                   num_devices=NCORES)
    # weights arrive sharded along the gate dim; AllGather on device
    featT = nc.dram_tensor("featT", [KA, R], bf16, kind="ExternalInput")
    wT_s = nc.dram_tensor("wT_s", [KA, NW // 8], bf16, kind="ExternalInput")
    whhf_s = nc.dram_tensor("whhf_s", [H, H4 // 8], bf16,
                            kind="ExternalInput")
    whhr_s = nc.dram_tensor("whhr_s", [H, H4 // 8], bf16,
                            kind="ExternalInput")
    linT = nc.dram_tensor("linT", [2 * H, NCLS], bf16, kind="ExternalInput")
    em = nc.dram_tensor("em", [R, NCLS], f32, kind="ExternalOutput")
    # collectives can't touch I/O tensors: bounce in, gather to Shared
    wT_b = nc.dram_tensor("wT_b", [KA, NW // 8], bf16)
    whhf_b = nc.dram_tensor("whhf_b", [H, H4 // 8], bf16)
    whhr_b = nc.dram_tensor("whhr_b", [H, H4 // 8], bf16)
    wT_g = nc.dram_tensor("wT_g", [NCORES * KA, NW // 8], bf16,
                          addr_space="Shared")
    whhf_g = nc.dram_tensor("whhf_g", [NCORES * H, H4 // 8], bf16,
                            addr_space="Shared")
    whhr_g = nc.dram_tensor("whhr_g", [NCORES * H, H4 // 8], bf16,
                            addr_space="Shared")
    GRP = [list(range(NCORES))]
    BYP = mybir.AluOpType.bypass

    ksz = [128, 128, KA - 256]
    ko = [0, 128, 256]

    with tile.TileContext(nc) as tc:
        with tc.tile_pool(name="wp", bufs=1) as wp, \
             tc.tile_pool(name="gxp", bufs=1) as gxp, \
             tc.tile_pool(name="stp", bufs=1) as stp, \
             tc.tile_pool(name="wk", bufs=3) as wk, \
             tc.tile_pool(name="pm", bufs=2, space="PSUM") as pm, \
             tc.tile_pool(name="pg", bufs=2, space="PSUM") as pg:

            # AllGather the weight shards (each core holds 1/8 of the gates)
            nc.sync.dma_start(wT_b[:, :], wT_s[:, :])
            nc.sync.dma_start(whhf_b[:, :], whhf_s[:, :])
            nc.sync.dma_start(whhr_b[:, :], whhr_s[:, :])
            nc.gpsimd.collective_compute("AllGather", BYP, GRP,
                                         ins=[wT_b[:, :]], outs=[wT_g[:, :]])
            nc.gpsimd.collective_compute("AllGather", BYP, GRP,
                                         ins=[whhf_b[:, :]],
                                         outs=[whhf_g[:, :]])
            nc.gpsimd.collective_compute("AllGather", BYP, GRP,
                                         ins=[whhr_b[:, :]],
                                         outs=[whhr_g[:, :]])

            # moving featT K-tiles and stationary wT K-tiles
            ft, wt = [], []
            for k in range(3):
                t1 = wp.tile([ksz[k], R], bf16, name=f"ft{k}", tag=f"ft{k}")
                nc.sync.dma_start(t1[:, :], featT[ko[k]:ko[k] + ksz[k], :])
                ft.append(t1)
                t2 = wp.tile([ksz[k], NW], bf16, name=f"wt{k}", tag=f"wt{k}")
                for c in range(NCORES):
                    nc.sync.dma_start(
                        t2[:, c * (NW // 8):(c + 1) * (NW // 8)],
                        wT_g[c * KA + ko[k]:c * KA + ko[k] + ksz[k], :])
                wt.append(t2)
            whh = {}
            for d, dram in ((0, whhf_g), (1, whhr_g)):
                t0 = wp.tile([128, H4], bf16, name=f"whh{d}0", tag=f"whh{d}0")
                t1 = wp.tile([128, H4], bf16, name=f"whh{d}1", tag=f"whh{d}1")
                for c in range(NCORES):
                    nc.sync.dma_start(
                        t0[:, c * (H4 // 8):(c + 1) * (H4 // 8)],
                        dram[c * H:c * H + 128, :])
                    nc.sync.dma_start(
                        t1[:, c * (H4 // 8):(c + 1) * (H4 // 8)],
                        dram[c * H + 128:c * H + 256, :])
                whh[d] = (t0, t1)
            lint = []
            for k in range(4):
                t = wp.tile([128, NCLS], bf16, name=f"lin{k}", tag=f"lin{k}")
                nc.sync.dma_start(t[:, :], linT[k * 128:(k + 1) * 128, :])
                lint.append(t)

            # gxT per direction: [128, S, 8, BC] fp32
            #   gxT_d[p, t, j, e] = gate (j*128+p) of dir d at time t, example e
            #   within-dir gate-slice order j: i0 i1 f0 f1 o0 o1 g0 g1
            gxT = [gxp.tile([128, S, 8, BC], f32, name=f"gxT{d}", tag=f"gxT{d}")
                   for d in range(2)]
            for d in range(2):
                for j in range(8):
                    gi = d * 8 + j
                    for rc in range(2):
                        ps = pm.tile([128, S // 2, BC], f32, tag="mm")
                        for k in range(3):
                            nc.tensor.matmul(
                                ps[:, :, :],
                                wt[k][:, gi * 128:(gi + 1) * 128],
                                ft[k][:, rc * 512:(rc + 1) * 512],
                                start=(k == 0), stop=(k == 2))
                        nc.scalar.copy(
                            gxT[d][:, rc * (S // 2):(rc + 1) * (S // 2), j, :],
                            ps[:, :, :])

            # hcat[0,1] = fwd h dims 0:128/128:256, hcat[2,3] = rev; col=t*8+e
            hcat = [stp.tile([128, R], bf16, name=f"hcat{j}", tag=f"hcat{j}")
                    for j in range(4)]
            # c state per dir: [128, 2, BC] (h dims 0:128 | 128:256)
            cst = [stp.tile([128, 2, BC], f32, name=f"c{d}", tag=f"c{d}")
                   for d in range(2)]

            from concourse.bass import ds

            def lstm_cell(d, gsl, first, hw_cols):
                """One LSTM cell update for direction d reading gates from
                gsl ([128,(1,)8,BC] pre-activation) and writing h to
                hcat[2d..2d+1][:, hw_cols]."""
                c_sb = cst[d]
                acts = wk.tile([128, 8, BC], f32, name=f"acts{d}",
                               tag=f"acts{d}")
                nc.scalar.activation(acts[:, 0:6, :], gsl[:, 0:6, :],
                                     AF.Sigmoid)
                nc.scalar.activation(acts[:, 6:8, :], gsl[:, 6:8, :], AF.Tanh)
                if first:
                    nc.vector.tensor_mul(c_sb[:, :, :], acts[:, 0:2, :],
                                         acts[:, 6:8, :])
                else:
                    fc = wk.tile([128, 2, BC], f32, name=f"fc{d}", tag=f"fc{d}")
                    nc.vector.tensor_mul(fc[:, :, :], acts[:, 2:4, :],
                                         c_sb[:, :, :])
                    nc.vector.tensor_mul(c_sb[:, :, :], acts[:, 0:2, :],
                                         acts[:, 6:8, :])
                    nc.vector.tensor_add(c_sb[:, :, :], c_sb[:, :, :],
                                         fc[:, :, :])
                th = wk.tile([128, 2, BC], f32, name=f"th{d}", tag=f"th{d}")
                nc.scalar.activation(th[:, :, :], c_sb[:, :, :], AF.Tanh)
                nc.vector.tensor_mul(hcat[2 * d][:, hw_cols],
                                     acts[:, 4, :], th[:, 0, :])
                nc.vector.tensor_mul(hcat[2 * d + 1][:, hw_cols],
                                     acts[:, 5, :], th[:, 1, :])

            # step 0 (no h feedback): gates come straight from gxT
            lstm_cell(0, gxT[0][:, 0], True, slice(0, BC))
            lstm_cell(1, gxT[1][:, S - 1], True, slice((S - 1) * BC, S * BC))

            # steps 1..S-1 as a hardware loop; sv = step index
            with tc.For_i(1, S, 1) as sv:
                for d in range(2):
                    # fwd: tt=sv prev=sv-1 ; rev: tt=S-1-sv prev=S-sv
                    if d == 0:
                        gcol = sv
                        pcol = sv * BC - BC
                        wcol = sv * BC
                    else:
                        gcol = S - 1 - sv
                        pcol = S * BC - sv * BC
                        wcol = (S - 1) * BC - sv * BC
                    ps = pg.tile([128, 8, BC], f32, name=f"rps{d}",
                                 tag=f"rps{d}")
                    w0, w1 = whh[d]
                    h0 = hcat[2 * d][:, ds(pcol, BC)]
                    h1 = hcat[2 * d + 1][:, ds(pcol, BC)]
                    for j in range(8):
                        nc.tensor.matmul(ps[:, j, :],
                                         w0[:, j * 128:(j + 1) * 128], h0,
                                         start=True, stop=False)
                        nc.tensor.matmul(ps[:, j, :],
                                         w1[:, j * 128:(j + 1) * 128], h1,
                                         start=False, stop=True)
                    g_sb = wk.tile([128, 8, BC], f32, name=f"g{d}",
                                   tag=f"g{d}")
                    nc.vector.tensor_add(g_sb[:, :, :], ps[:, :, :],
                                         gxT[d][:, ds(gcol, 1)])
                    lstm_cell(d, g_sb, False, ds(wcol, BC))

            for m in range(8):
                pe = pm.tile([128, NCLS], f32, tag="mm")
                for k in range(4):
                    nc.tensor.matmul(pe[:, :], hcat[k][:, m * 128:(m + 1) * 128],
                                     lint[k][:, :], start=(k == 0),
                                     stop=(k == 3))
                eo = wk.tile([128, NCLS], f32, tag="emo")
                nc.any.tensor_copy(eo[:, :], pe[:, :])
                nc.sync.dma_start(em[m * 128:(m + 1) * 128, :], eo[:, :])
    nc.compile()
    return nc


_NC_CACHE = {}
LAST_DEVICE_NS = [0]


def _run_device(in_maps):
    import time
    from concourse.bass_utils import run_bass_kernel_spmd
    if "nc" not in _NC_CACHE:
        _NC_CACHE["nc"] = _build_nc()
    nc = _NC_CACHE["nc"]
    t0 = time.time()
    res = run_bass_kernel_spmd(nc, in_maps, core_ids=list(range(NCORES)))
    LAST_DEVICE_NS[0] = int((time.time() - t0) * 1e9)
    return [r["em"] for r in res.results]


def _logsumexp(x, axis):
    m = np.max(x, axis=axis, keepdims=True)
    return (m + np.log(np.sum(np.exp(x - m), axis=axis,
                              keepdims=True))).squeeze(axis)


def kernel(word_table, char_table, conv_w, conv_b, w_ih_f, w_hh_f, b_f,
           w_ih_r, w_hh_r, b_r, lin_w, lin_b, start_t, end_t, trans,
           sent, word, tag, mask):
    import ml_dtypes
    bf = ml_dtypes.bfloat16
    word_table = np.asarray(word_table, np.float32)
    char_table = np.asarray(char_table, np.float32)
    conv_w = np.asarray(conv_w, np.float32)
    conv_b = np.asarray(conv_b, np.float32)
    lin_w = np.asarray(lin_w, np.float32)
    lin_b = np.asarray(lin_b, np.float32)
    start_t = np.asarray(start_t, np.float32)
    end_t = np.asarray(end_t, np.float32)
    trans = np.asarray(trans, np.float32)
    sent_i = np.asarray(sent).astype(np.int64)
    word_i = np.asarray(word).astype(np.int64)
    tag_i = np.asarray(tag).astype(np.int64)
    mask_b = np.asarray(mask).astype(bool)

    # --- char CNN (host: tiny) ---
    ct = char_table.copy()
    ct[0] = 0.0
    cemb = ct[word_i.reshape(-1)].reshape(B * S, LW, CHAR_E)
    pad = np.zeros((B * S, LW + 2, CHAR_E), np.float32)
    pad[:, 1:LW + 1, :] = cemb
    conv = np.zeros((B * S, LW, CHAR_C), np.float32)
    for dk in range(3):
        conv += pad[:, dk:dk + LW, :] @ conv_w[:, :, dk].T
    conv += conv_b[None, None, :]
    char_feat = conv.max(axis=1).reshape(B, S, CHAR_C)

    # --- word embedding + concat ---
    wemb = word_table[sent_i.reshape(-1)].reshape(B, S, WORD_E)
    feat = np.concatenate([wemb, char_feat], axis=2)  # [B,S,F]

    # --- device: projections + biLSTM + linear -> emissions ---
    gp = _GATE_PERM
    wcat = np.concatenate([np.asarray(w_ih_f, np.float32)[gp],
                           np.asarray(w_ih_r, np.float32)[gp]], axis=0)
    bcat = np.concatenate([np.asarray(b_f, np.float32)[gp],
                           np.asarray(b_r, np.float32)[gp]])
    wT = np.empty((KA, NW), np.float32)
    wT[:F] = wcat.T
    wT[F] = bcat
    wT = wT.astype(bf)
    whhT_f_a = np.ascontiguousarray(
        np.asarray(w_hh_f, np.float32)[gp].T).astype(bf)
    whhT_r_a = np.ascontiguousarray(
        np.asarray(w_hh_r, np.float32)[gp].T).astype(bf)
    linT = np.ascontiguousarray(lin_w.T).astype(bf)

    in_maps = []
    for c in range(NCORES):
        fc = feat[c * BC:(c + 1) * BC]  # [BC,S,F]
        fT = np.empty((KA, R), np.float32)
        fT[:F] = fc.transpose(2, 1, 0).reshape(F, R)
        fT[F] = 1.0
        gs, hs = NW // 8, H4 // 8
        in_maps.append(dict(
            featT=fT.astype(bf),
            wT_s=np.ascontiguousarray(wT[:, c * gs:(c + 1) * gs]),
            whhf_s=np.ascontiguousarray(whhT_f_a[:, c * hs:(c + 1) * hs]),
            whhr_s=np.ascontiguousarray(whhT_r_a[:, c * hs:(c + 1) * hs]),
            linT=linT))
    em_shards = _run_device(in_maps)
    em = np.empty((S, B, NCLS), np.float32)
    for c in range(NCORES):
        em[:, c * BC:(c + 1) * BC, :] = em_shards[c].reshape(S, BC, NCLS)
    em += lin_b[None, None, :]

    # --- CRF NLL (host) ---
    tg = tag_i.T  # [S,B]
    mk = mask_b.T.astype(np.float32)
    bidx = np.arange(B)
    em_tag = np.take_along_axis(em, tg[..., None], axis=-1)[..., 0]
    tr = trans[tg[:-1], tg[1:]]
    score = start_t[tg[0]] + em_tag[0] + np.sum(
        mk[1:] * (tr + em_tag[1:]), axis=0)
    last = mk.sum(0).astype(np.int64) - 1
    score = score + end_t[tg[last, bidx]]
    alpha = start_t[None, :] + em[0]
    for t in range(1, S):
        nxt = _logsumexp(
            alpha[:, :, None] + trans[None, :, :] + em[t][:, None, :], axis=1)
        alpha = np.where(mk[t][:, None] > 0, nxt, alpha)
    logZ = _logsumexp(alpha + end_t[None, :], axis=1)
    return np.asarray(-np.sum(score - logZ), np.float32)


# revision 14
# speedup vs baseline: 10.2501x; 1.2539x over previous
import numpy as np

# CNN-biLSTM-CRF forward NLL, data-parallel over batch across 8 NeuronCores.
# Device computes the input projections, the full biLSTM recurrence and the
# emission linear layer; host handles embedding gathers, the tiny char-CNN
# and the CRF scan. Only the [1024,25] emissions per core come back, so the
# axon transfer (the dominant cost) is ~26x smaller than shipping gates.

B, S, LW = 64, 128, 20
CHAR_E, CHAR_C = 30, 30
WORD_E = 300
H, NCLS = 256, 25
F = WORD_E + CHAR_C  # 330
KA = F + 1  # ones row folds the gate bias into the matmul
NCORES = 8
BC = B // NCORES  # 8 examples per core
R = BC * S  # 1024 rows per core, time-major: row = t*BC + e
NW = 8 * H  # 2048 = both directions' 4H gates
H4 = 4 * H

# gate order on device: i, f, o, g  (sigmoid on [0:3H), tanh on [3H:4H))
_GATE_PERM = np.concatenate([
    np.arange(0, H),            # i
    np.arange(H, 2 * H),        # f
    np.arange(3 * H, 4 * H),    # o
    np.arange(2 * H, 3 * H),    # g
])


def _build_nc():
    import concourse.bacc as bacc
    import concourse.mybir as mybir
    from concourse import tile

    f32 = mybir.dt.float32
    bf16 = mybir.dt.bfloat16
    f8 = mybir.dt.float8e4
    AF = mybir.ActivationFunctionType

    nc = bacc.Bacc("TRN2", target_bir_lowering=False, debug=False,
                   num_devices=NCORES)
    # weights arrive sharded along the gate dim; AllGather on device
    featT = nc.dram_tensor("featT", [KA, R], f8, kind="ExternalInput")
    wT_s = nc.dram_tensor("wT_s", [KA, NW // 8], f8, kind="ExternalInput")
    whhf_s = nc.dram_tensor("whhf_s", [H, H4 // 8], bf16,
                            kind="ExternalInput")
    whhr_s = nc.dram_tensor("whhr_s", [H, H4 // 8], bf16,
                            kind="ExternalInput")
    linT = nc.dram_tensor("linT", [2 * H, NCLS], bf16, kind="ExternalInput")
    em = nc.dram_tensor("em", [R, NCLS], bf16, kind="ExternalOutput")
    # collectives can't touch I/O tensors: bounce in, gather to Shared
    wT_b = nc.dram_tensor("wT_b", [KA, NW // 8], f8)
    whhf_b = nc.dram_tensor("whhf_b", [H, H4 // 8], bf16)
    whhr_b = nc.dram_tensor("whhr_b", [H, H4 // 8], bf16)
    wT_g = nc.dram_tensor("wT_g", [NCORES * KA, NW // 8], f8,
                          addr_space="Shared")
    whhf_g = nc.dram_tensor("whhf_g", [NCORES * H, H4 // 8], bf16,
                            addr_space="Shared")
    whhr_g = nc.dram_tensor("whhr_g", [NCORES * H, H4 // 8], bf16,
                            addr_space="Shared")
    GRP = [list(range(NCORES))]
    BYP = mybir.AluOpType.bypass

    ksz = [128, 128, KA - 256]
    ko = [0, 128, 256]

    with tile.TileContext(nc) as tc:
        with tc.tile_pool(name="wp", bufs=1) as wp, \
             tc.tile_pool(name="gxp", bufs=1) as gxp, \
             tc.tile_pool(name="stp", bufs=1) as stp, \
             tc.tile_pool(name="wk", bufs=3) as wk, \
             tc.tile_pool(name="pm", bufs=2, space="PSUM") as pm, \
             tc.tile_pool(name="pg", bufs=2, space="PSUM") as pg:

            # AllGather the weight shards (each core holds 1/8 of the gates)
            nc.sync.dma_start(wT_b[:, :], wT_s[:, :])
            nc.sync.dma_start(whhf_b[:, :], whhf_s[:, :])
            nc.sync.dma_start(whhr_b[:, :], whhr_s[:, :])
            nc.gpsimd.collective_compute("AllGather", BYP, GRP,
                                         ins=[wT_b[:, :]], outs=[wT_g[:, :]])
            nc.gpsimd.collective_compute("AllGather", BYP, GRP,
                                         ins=[whhf_b[:, :]],
                                         outs=[whhf_g[:, :]])
            nc.gpsimd.collective_compute("AllGather", BYP, GRP,
                                         ins=[whhr_b[:, :]],
                                         outs=[whhr_g[:, :]])

            # moving featT K-tiles and stationary wT K-tiles
            ft, wt = [], []
            for k in range(3):
                t1 = wp.tile([ksz[k], R], f8, name=f"ft{k}", tag=f"ft{k}")
                nc.sync.dma_start(t1[:, :], featT[ko[k]:ko[k] + ksz[k], :])
                ft.append(t1)
                t2 = wp.tile([ksz[k], NW], f8, name=f"wt{k}", tag=f"wt{k}")
                for c in range(NCORES):
                    nc.sync.dma_start(
                        t2[:, c * (NW // 8):(c + 1) * (NW // 8)],
                        wT_g[c * KA + ko[k]:c * KA + ko[k] + ksz[k], :])
                wt.append(t2)
            whh = {}
            for d, dram in ((0, whhf_g), (1, whhr_g)):
                t0 = wp.tile([128, H4], bf16, name=f"whh{d}0", tag=f"whh{d}0")
                t1 = wp.tile([128, H4], bf16, name=f"whh{d}1", tag=f"whh{d}1")
                for c in range(NCORES):
                    nc.sync.dma_start(
                        t0[:, c * (H4 // 8):(c + 1) * (H4 // 8)],
                        dram[c * H:c * H + 128, :])
                    nc.sync.dma_start(
                        t1[:, c * (H4 // 8):(c + 1) * (H4 // 8)],
                        dram[c * H + 128:c * H + 256, :])
                whh[d] = (t0, t1)
            lint = []
            for k in range(4):
                t = wp.tile([128, NCLS], bf16, name=f"lin{k}", tag=f"lin{k}")
                nc.sync.dma_start(t[:, :], linT[k * 128:(k + 1) * 128, :])
                lint.append(t)

            # gxT per direction: [128, S, 8, BC] fp32
            #   gxT_d[p, t, j, e] = gate (j*128+p) of dir d at time t, example e
            #   within-dir gate-slice order j: i0 i1 f0 f1 o0 o1 g0 g1
            gxT = [gxp.tile([128, S, 8, BC], f32, name=f"gxT{d}", tag=f"gxT{d}")
                   for d in range(2)]
            for d in range(2):
                for j in range(8):
                    gi = d * 8 + j
                    for rc in range(2):
                        ps = pm.tile([128, S // 2, BC], f32, tag="mm")
                        for k in range(3):
                            nc.tensor.matmul(
                                ps[:, :, :],
                                wt[k][:, gi * 128:(gi + 1) * 128],
                                ft[k][:, rc * 512:(rc + 1) * 512],
                                start=(k == 0), stop=(k == 2))
                        nc.scalar.copy(
                            gxT[d][:, rc * (S // 2):(rc + 1) * (S // 2), j, :],
                            ps[:, :, :])

            # hcat[0,1] = fwd h dims 0:128/128:256, hcat[2,3] = rev; col=t*8+e
            hcat = [stp.tile([128, R], bf16, name=f"hcat{j}", tag=f"hcat{j}")
                    for j in range(4)]
            # c state per dir: [128, 2, BC] (h dims 0:128 | 128:256)
            cst = [stp.tile([128, 2, BC], f32, name=f"c{d}", tag=f"c{d}")
                   for d in range(2)]

            from concourse.bass import ds

            def lstm_cell(d, gsl, first, hw_cols):
                """One LSTM cell update for direction d reading gates from
                gsl ([128,(1,)8,BC] pre-activation) and writing h to
                hcat[2d..2d+1][:, hw_cols]."""
                c_sb = cst[d]
                acts = wk.tile([128, 8, BC], f32, name=f"acts{d}",
                               tag=f"acts{d}")
                nc.scalar.activation(acts[:, 0:6, :], gsl[:, 0:6, :],
                                     AF.Sigmoid)
                nc.scalar.activation(acts[:, 6:8, :], gsl[:, 6:8, :], AF.Tanh)
                if first:
                    nc.vector.tensor_mul(c_sb[:, :, :], acts[:, 0:2, :],
                                         acts[:, 6:8, :])
                else:
                    fc = wk.tile([128, 2, BC], f32, name=f"fc{d}", tag=f"fc{d}")
                    nc.vector.tensor_mul(fc[:, :, :], acts[:, 2:4, :],
                                         c_sb[:, :, :])
                    nc.vector.tensor_mul(c_sb[:, :, :], acts[:, 0:2, :],
                                         acts[:, 6:8, :])
                    nc.vector.tensor_add(c_sb[:, :, :], c_sb[:, :, :],
                                         fc[:, :, :])
                th = wk.tile([128, 2, BC], f32, name=f"th{d}", tag=f"th{d}")
                nc.scalar.activation(th[:, :, :], c_sb[:, :, :], AF.Tanh)
                nc.vector.tensor_mul(hcat[2 * d][:, hw_cols],
                                     acts[:, 4, :], th[:, 0, :])
                nc.vector.tensor_mul(hcat[2 * d + 1][:, hw_cols],
                                     acts[:, 5, :], th[:, 1, :])

            # step 0 (no h feedback): gates come straight from gxT
            lstm_cell(0, gxT[0][:, 0], True, slice(0, BC))
            lstm_cell(1, gxT[1][:, S - 1], True, slice((S - 1) * BC, S * BC))

            # steps 1..S-1 as a hardware loop; sv = step index
            with tc.For_i(1, S, 1) as sv:
                for d in range(2):
                    # fwd: tt=sv prev=sv-1 ; rev: tt=S-1-sv prev=S-sv
                    if d == 0:
                        gcol = sv
                        pcol = sv * BC - BC
                        wcol = sv * BC
                    else:
                        gcol = S - 1 - sv
                        pcol = S * BC - sv * BC
                        wcol = (S - 1) * BC - sv * BC
                    ps = pg.tile([128, 8, BC], f32, name=f"rps{d}",
                                 tag=f"rps{d}")
                    w0, w1 = whh[d]
                    h0 = hcat[2 * d][:, ds(pcol, BC)]
                    h1 = hcat[2 * d + 1][:, ds(pcol, BC)]
                    for j in range(8):
                        nc.tensor.matmul(ps[:, j, :],
                                         w0[:, j * 128:(j + 1) * 128], h0,
                                         start=True, stop=False)
                        nc.tensor.matmul(ps[:, j, :],
                                         w1[:, j * 128:(j + 1) * 128], h1,
                                         start=False, stop=True)
                    g_sb = wk.tile([128, 8, BC], f32, name=f"g{d}",
                                   tag=f"g{d}")
                    nc.vector.tensor_add(g_sb[:, :, :], ps[:, :, :],
                                         gxT[d][:, ds(gcol, 1)])
                    lstm_cell(d, g_sb, False, ds(wcol, BC))

            for m in range(8):
                pe = pm.tile([128, NCLS], f32, tag="mm")
                for k in range(4):
                    nc.tensor.matmul(pe[:, :], hcat[k][:, m * 128:(m + 1) * 128],
                                     lint[k][:, :], start=(k == 0),
                                     stop=(k == 3))
                eo = wk.tile([128, NCLS], bf16, tag="emo")
                nc.any.tensor_copy(eo[:, :], pe[:, :])
                nc.sync.dma_start(em[m * 128:(m + 1) * 128, :], eo[:, :])
    nc.compile()
    return nc


_NC_CACHE = {}
LAST_DEVICE_NS = [0]


def _run_device(in_maps):
    import time
    from concourse.bass_utils import run_bass_kernel_spmd
    if "nc" not in _NC_CACHE:
        _NC_CACHE["nc"] = _build_nc()
    nc = _NC_CACHE["nc"]
    t0 = time.time()
    res = run_bass_kernel_spmd(nc, in_maps, core_ids=list(range(NCORES)))
    LAST_DEVICE_NS[0] = int((time.time() - t0) * 1e9)
    return [r["em"] for r in res.results]


def _logsumexp(x, axis):
    m = np.max(x, axis=axis, keepdims=True)
    return (m + np.log(np.sum(np.exp(x - m), axis=axis,
                              keepdims=True))).squeeze(axis)


def kernel(word_table, char_table, conv_w, conv_b, w_ih_f, w_hh_f, b_f,
           w_ih_r, w_hh_r, b_r, lin_w, lin_b, start_t, end_t, trans,
           sent, word, tag, mask):
    import ml_dtypes
    bf = ml_dtypes.bfloat16
    f8 = ml_dtypes.float8_e4m3
    word_table = np.asarray(word_table, np.float32)
    char_table = np.asarray(char_table, np.float32)
    conv_w = np.asarray(conv_w, np.float32)
    conv_b = np.asarray(conv_b, np.float32)
    lin_w = np.asarray(lin_w, np.float32)
    lin_b = np.asarray(lin_b, np.float32)
    start_t = np.asarray(start_t, np.float32)
    end_t = np.asarray(end_t, np.float32)
    trans = np.asarray(trans, np.float32)
    sent_i = np.asarray(sent).astype(np.int64)
    word_i = np.asarray(word).astype(np.int64)
    tag_i = np.asarray(tag).astype(np.int64)
    mask_b = np.asarray(mask).astype(bool)

    # --- char CNN (host: tiny) ---
    ct = char_table.copy()
    ct[0] = 0.0
    cemb = ct[word_i.reshape(-1)].reshape(B * S, LW, CHAR_E)
    pad = np.zeros((B * S, LW + 2, CHAR_E), np.float32)
    pad[:, 1:LW + 1, :] = cemb
    conv = np.zeros((B * S, LW, CHAR_C), np.float32)
    for dk in range(3):
        conv += pad[:, dk:dk + LW, :] @ conv_w[:, :, dk].T
    conv += conv_b[None, None, :]
    char_feat = conv.max(axis=1).reshape(B, S, CHAR_C)

    # --- word embedding + concat ---
    wemb = word_table[sent_i.reshape(-1)].reshape(B, S, WORD_E)
    feat = np.concatenate([wemb, char_feat], axis=2)  # [B,S,F]

    # --- device: projections + biLSTM + linear -> emissions ---
    gp = _GATE_PERM
    wcat = np.concatenate([np.asarray(w_ih_f, np.float32)[gp],
                           np.asarray(w_ih_r, np.float32)[gp]], axis=0)
    bcat = np.concatenate([np.asarray(b_f, np.float32)[gp],
                           np.asarray(b_r, np.float32)[gp]])
    wT = np.empty((KA, NW), np.float32)
    wT[:F] = wcat.T
    wT[F] = bcat
    wT = wT.astype(f8)
    whhT_f_a = np.ascontiguousarray(
        np.asarray(w_hh_f, np.float32)[gp].T).astype(bf)
    whhT_r_a = np.ascontiguousarray(
        np.asarray(w_hh_r, np.float32)[gp].T).astype(bf)
    linT = np.ascontiguousarray(lin_w.T).astype(bf)

    in_maps = []
    for c in range(NCORES):
        fc = feat[c * BC:(c + 1) * BC]  # [BC,S,F]
        fT = np.empty((KA, R), np.float32)
        fT[:F] = fc.transpose(2, 1, 0).reshape(F, R)
        fT[F] = 1.0
        gs, hs = NW // 8, H4 // 8
        in_maps.append(dict(
            featT=fT.astype(f8),
            wT_s=np.ascontiguousarray(wT[:, c * gs:(c + 1) * gs]),
            whhf_s=np.ascontiguousarray(whhT_f_a[:, c * hs:(c + 1) * hs]),
            whhr_s=np.ascontiguousarray(whhT_r_a[:, c * hs:(c + 1) * hs]),
            linT=linT))
    em_shards = _run_device(in_maps)
    em = np.empty((S, B, NCLS), np.float32)
    for c in range(NCORES):
        em[:, c * BC:(c + 1) * BC, :] = (
            em_shards[c].astype(np.float32).reshape(S, BC, NCLS))
    em += lin_b[None, None, :]

    # --- CRF NLL (host) ---
    tg = tag_i.T  # [S,B]
    mk = mask_b.T.astype(np.float32)
    bidx = np.arange(B)
    em_tag = np.take_along_axis(em, tg[..., None], axis=-1)[..., 0]
    tr = trans[tg[:-1], tg[1:]]
    score = start_t[tg[0]] + em_tag[0] + np.sum(
        mk[1:] * (tr + em_tag[1:]), axis=0)
    last = mk.sum(0).astype(np.int64) - 1
    score = score + end_t[tg[last, bidx]]
    alpha = start_t[None, :] + em[0]
    for t in range(1, S):
        nxt = _logsumexp(
            alpha[:, :, None] + trans[None, :, :] + em[t][:, None, :], axis=1)
        alpha = np.where(mk[t][:, None] > 0, nxt, alpha)
    logZ = _logsumexp(alpha + end_t[None, :], axis=1)
    return np.asarray(-np.sum(score - logZ), np.float32)


# revision 15
# speedup vs baseline: 13.9656x; 1.3625x over previous
import numpy as np

# Persistent XLA compilation cache: run_bass_kernel_spmd re-traces its jit
# closure every call, so without this each call pays a full XLA re-compile.
try:
    import jax
    jax.config.update("jax_compilation_cache_dir", "/tmp/jax_comp_cache")
    jax.config.update("jax_persistent_cache_min_entry_size_bytes", -1)
    jax.config.update("jax_persistent_cache_min_compile_time_secs", 0.0)
except Exception:
    pass

# CNN-biLSTM-CRF forward NLL, data-parallel over batch across 8 NeuronCores.
# Device computes the input projections, the full biLSTM recurrence and the
# emission linear layer; host handles embedding gathers, the tiny char-CNN
# and the CRF scan. Only the [1024,25] emissions per core come back, so the
# axon transfer (the dominant cost) is ~26x smaller than shipping gates.

B, S, LW = 64, 128, 20
CHAR_E, CHAR_C = 30, 30
WORD_E = 300
H, NCLS = 256, 25
F = WORD_E + CHAR_C  # 330
KA = F + 1  # ones row folds the gate bias into the matmul
NCORES = 8
BC = B // NCORES  # 8 examples per core
R = BC * S  # 1024 rows per core, time-major: row = t*BC + e
NW = 8 * H  # 2048 = both directions' 4H gates
H4 = 4 * H

# gate order on device: i, f, o, g  (sigmoid on [0:3H), tanh on [3H:4H))
_GATE_PERM = np.concatenate([
    np.arange(0, H),            # i
    np.arange(H, 2 * H),        # f
    np.arange(3 * H, 4 * H),    # o
    np.arange(2 * H, 3 * H),    # g
])


def _build_nc():
    import concourse.bacc as bacc
    import concourse.mybir as mybir
    from concourse import tile

    f32 = mybir.dt.float32
    bf16 = mybir.dt.bfloat16
    f8 = mybir.dt.float8e4
    AF = mybir.ActivationFunctionType

    nc = bacc.Bacc("TRN2", target_bir_lowering=False, debug=False,
                   num_devices=NCORES)
    # weights arrive sharded along the gate dim; AllGather on device
    featT = nc.dram_tensor("featT", [KA, R], f8, kind="ExternalInput")
    wT_s = nc.dram_tensor("wT_s", [KA, NW // 8], f8, kind="ExternalInput")
    whhf_s = nc.dram_tensor("whhf_s", [H, H4 // 8], bf16,
                            kind="ExternalInput")
    whhr_s = nc.dram_tensor("whhr_s", [H, H4 // 8], bf16,
                            kind="ExternalInput")
    linT = nc.dram_tensor("linT", [2 * H, NCLS], bf16, kind="ExternalInput")
    em = nc.dram_tensor("em", [R, NCLS], bf16, kind="ExternalOutput")
    # collectives can't touch I/O tensors: bounce in, gather to Shared
    wT_b = nc.dram_tensor("wT_b", [KA, NW // 8], f8)
    whhf_b = nc.dram_tensor("whhf_b", [H, H4 // 8], bf16)
    whhr_b = nc.dram_tensor("whhr_b", [H, H4 // 8], bf16)
    wT_g = nc.dram_tensor("wT_g", [NCORES * KA, NW // 8], f8,
                          addr_space="Shared")
    whhf_g = nc.dram_tensor("whhf_g", [NCORES * H, H4 // 8], bf16,
                            addr_space="Shared")
    whhr_g = nc.dram_tensor("whhr_g", [NCORES * H, H4 // 8], bf16,
                            addr_space="Shared")
    GRP = [list(range(NCORES))]
    BYP = mybir.AluOpType.bypass

    ksz = [128, 128, KA - 256]
    ko = [0, 128, 256]

    with tile.TileContext(nc) as tc:
        with tc.tile_pool(name="wp", bufs=1) as wp, \
             tc.tile_pool(name="gxp", bufs=1) as gxp, \
             tc.tile_pool(name="stp", bufs=1) as stp, \
             tc.tile_pool(name="wk", bufs=3) as wk, \
             tc.tile_pool(name="pm", bufs=2, space="PSUM") as pm, \
             tc.tile_pool(name="pg", bufs=2, space="PSUM") as pg:

            # AllGather the weight shards (each core holds 1/8 of the gates)
            nc.sync.dma_start(wT_b[:, :], wT_s[:, :])
            nc.sync.dma_start(whhf_b[:, :], whhf_s[:, :])
            nc.sync.dma_start(whhr_b[:, :], whhr_s[:, :])
            nc.gpsimd.collective_compute("AllGather", BYP, GRP,
                                         ins=[wT_b[:, :]], outs=[wT_g[:, :]])
            nc.gpsimd.collective_compute("AllGather", BYP, GRP,
                                         ins=[whhf_b[:, :]],
                                         outs=[whhf_g[:, :]])
            nc.gpsimd.collective_compute("AllGather", BYP, GRP,
                                         ins=[whhr_b[:, :]],
                                         outs=[whhr_g[:, :]])

            # moving featT K-tiles and stationary wT K-tiles
            ft, wt = [], []
            for k in range(3):
                t1 = wp.tile([ksz[k], R], f8, name=f"ft{k}", tag=f"ft{k}")
                nc.sync.dma_start(t1[:, :], featT[ko[k]:ko[k] + ksz[k], :])
                ft.append(t1)
                t2 = wp.tile([ksz[k], NW], f8, name=f"wt{k}", tag=f"wt{k}")
                for c in range(NCORES):
                    nc.sync.dma_start(
                        t2[:, c * (NW // 8):(c + 1) * (NW // 8)],
                        wT_g[c * KA + ko[k]:c * KA + ko[k] + ksz[k], :])
                wt.append(t2)
            whh = {}
            for d, dram in ((0, whhf_g), (1, whhr_g)):
                t0 = wp.tile([128, H4], bf16, name=f"whh{d}0", tag=f"whh{d}0")
                t1 = wp.tile([128, H4], bf16, name=f"whh{d}1", tag=f"whh{d}1")
                for c in range(NCORES):
                    nc.sync.dma_start(
                        t0[:, c * (H4 // 8):(c + 1) * (H4 // 8)],
                        dram[c * H:c * H + 128, :])
                    nc.sync.dma_start(
                        t1[:, c * (H4 // 8):(c + 1) * (H4 // 8)],
                        dram[c * H + 128:c * H + 256, :])
                whh[d] = (t0, t1)
            lint = []
            for k in range(4):
                t = wp.tile([128, NCLS], bf16, name=f"lin{k}", tag=f"lin{k}")
                nc.sync.dma_start(t[:, :], linT[k * 128:(k + 1) * 128, :])
                lint.append(t)

            # gxT per direction: [128, S, 8, BC] fp32
            #   gxT_d[p, t, j, e] = gate (j*128+p) of dir d at time t, example e
            #   within-dir gate-slice order j: i0 i1 f0 f1 o0 o1 g0 g1
            gxT = [gxp.tile([128, S, 8, BC], f32, name=f"gxT{d}", tag=f"gxT{d}")
                   for d in range(2)]
            for d in range(2):
                for j in range(8):
                    gi = d * 8 + j
                    for rc in range(2):
                        ps = pm.tile([128, S // 2, BC], f32, tag="mm")
                        for k in range(3):
                            nc.tensor.matmul(
                                ps[:, :, :],
                                wt[k][:, gi * 128:(gi + 1) * 128],
                                ft[k][:, rc * 512:(rc + 1) * 512],
                                start=(k == 0), stop=(k == 2))
                        nc.scalar.copy(
                            gxT[d][:, rc * (S // 2):(rc + 1) * (S // 2), j, :],
                            ps[:, :, :])

            # hcat[0,1] = fwd h dims 0:128/128:256, hcat[2,3] = rev; col=t*8+e
            hcat = [stp.tile([128, R], bf16, name=f"hcat{j}", tag=f"hcat{j}")
                    for j in range(4)]
            # c state per dir: [128, 2, BC] (h dims 0:128 | 128:256)
            cst = [stp.tile([128, 2, BC], f32, name=f"c{d}", tag=f"c{d}")
                   for d in range(2)]

            from concourse.bass import ds

            def lstm_cell(d, gsl, first, hw_cols):
                """One LSTM cell update for direction d reading gates from
                gsl ([128,(1,)8,BC] pre-activation) and writing h to
                hcat[2d..2d+1][:, hw_cols]."""
                c_sb = cst[d]
                acts = wk.tile([128, 8, BC], f32, name=f"acts{d}",
                               tag=f"acts{d}")
                nc.scalar.activation(acts[:, 0:6, :], gsl[:, 0:6, :],
                                     AF.Sigmoid)
                nc.scalar.activation(acts[:, 6:8, :], gsl[:, 6:8, :], AF.Tanh)
                if first:
                    nc.vector.tensor_mul(c_sb[:, :, :], acts[:, 0:2, :],
                                         acts[:, 6:8, :])
                else:
                    fc = wk.tile([128, 2, BC], f32, name=f"fc{d}", tag=f"fc{d}")
                    nc.vector.tensor_mul(fc[:, :, :], acts[:, 2:4, :],
                                         c_sb[:, :, :])
                    nc.vector.tensor_mul(c_sb[:, :, :], acts[:, 0:2, :],
                                         acts[:, 6:8, :])
                    nc.vector.tensor_add(c_sb[:, :, :], c_sb[:, :, :],
                                         fc[:, :, :])
                th = wk.tile([128, 2, BC], f32, name=f"th{d}", tag=f"th{d}")
                nc.scalar.activation(th[:, :, :], c_sb[:, :, :], AF.Tanh)
                nc.vector.tensor_mul(hcat[2 * d][:, hw_cols],
                                     acts[:, 4, :], th[:, 0, :])
                nc.vector.tensor_mul(hcat[2 * d + 1][:, hw_cols],
                                     acts[:, 5, :], th[:, 1, :])

            # step 0 (no h feedback): gates come straight from gxT
            lstm_cell(0, gxT[0][:, 0], True, slice(0, BC))
            lstm_cell(1, gxT[1][:, S - 1], True, slice((S - 1) * BC, S * BC))

            # steps 1..S-1 as a hardware loop; sv = step index
            with tc.For_i(1, S, 1) as sv:
                for d in range(2):
                    # fwd: tt=sv prev=sv-1 ; rev: tt=S-1-sv prev=S-sv
                    if d == 0:
                        gcol = sv
                        pcol = sv * BC - BC
                        wcol = sv * BC
                    else:
                        gcol = S - 1 - sv
                        pcol = S * BC - sv * BC
                        wcol = (S - 1) * BC - sv * BC
                    ps = pg.tile([128, 8, BC], f32, name=f"rps{d}",
                                 tag=f"rps{d}")
                    w0, w1 = whh[d]
                    h0 = hcat[2 * d][:, ds(pcol, BC)]
                    h1 = hcat[2 * d + 1][:, ds(pcol, BC)]
                    for j in range(8):
                        nc.tensor.matmul(ps[:, j, :],
                                         w0[:, j * 128:(j + 1) * 128], h0,
                                         start=True, stop=False)
                        nc.tensor.matmul(ps[:, j, :],
                                         w1[:, j * 128:(j + 1) * 128], h1,
                                         start=False, stop=True)
                    g_sb = wk.tile([128, 8, BC], f32, name=f"g{d}",
                                   tag=f"g{d}")
                    nc.vector.tensor_add(g_sb[:, :, :], ps[:, :, :],
                                         gxT[d][:, ds(gcol, 1)])
                    lstm_cell(d, g_sb, False, ds(wcol, BC))

            for m in range(8):
                pe = pm.tile([128, NCLS], f32, tag="mm")
                for k in range(4):
                    nc.tensor.matmul(pe[:, :], hcat[k][:, m * 128:(m + 1) * 128],
                                     lint[k][:, :], start=(k == 0),
                                     stop=(k == 3))
                eo = wk.tile([128, NCLS], bf16, tag="emo")
                nc.any.tensor_copy(eo[:, :], pe[:, :])
                nc.sync.dma_start(em[m * 128:(m + 1) * 128, :], eo[:, :])
    nc.compile()
    return nc


_NC_CACHE = {}
LAST_DEVICE_NS = [0]


def _run_device(in_maps):
    import time
    from concourse.bass_utils import run_bass_kernel_spmd
    if "nc" not in _NC_CACHE:
        _NC_CACHE["nc"] = _build_nc()
    nc = _NC_CACHE["nc"]
    t0 = time.time()
    res = run_bass_kernel_spmd(nc, in_maps, core_ids=list(range(NCORES)))
    LAST_DEVICE_NS[0] = int((time.time() - t0) * 1e9)
    return [r["em"] for r in res.results]


def _logsumexp(x, axis):
    m = np.max(x, axis=axis, keepdims=True)
    return (m + np.log(np.sum(np.exp(x - m), axis=axis,
                              keepdims=True))).squeeze(axis)


def kernel(word_table, char_table, conv_w, conv_b, w_ih_f, w_hh_f, b_f,
           w_ih_r, w_hh_r, b_r, lin_w, lin_b, start_t, end_t, trans,
           sent, word, tag, mask):
    import ml_dtypes
    bf = ml_dtypes.bfloat16
    f8 = ml_dtypes.float8_e4m3
    word_table = np.asarray(word_table, np.float32)
    char_table = np.asarray(char_table, np.float32)
    conv_w = np.asarray(conv_w, np.float32)
    conv_b = np.asarray(conv_b, np.float32)
    lin_w = np.asarray(lin_w, np.float32)
    lin_b = np.asarray(lin_b, np.float32)
    start_t = np.asarray(start_t, np.float32)
    end_t = np.asarray(end_t, np.float32)
    trans = np.asarray(trans, np.float32)
    sent_i = np.asarray(sent).astype(np.int64)
    word_i = np.asarray(word).astype(np.int64)
    tag_i = np.asarray(tag).astype(np.int64)
    mask_b = np.asarray(mask).astype(bool)

    # --- char CNN (host: tiny) ---
    ct = char_table.copy()
    ct[0] = 0.0
    cemb = ct[word_i.reshape(-1)].reshape(B * S, LW, CHAR_E)
    pad = np.zeros((B * S, LW + 2, CHAR_E), np.float32)
    pad[:, 1:LW + 1, :] = cemb
    conv = np.zeros((B * S, LW, CHAR_C), np.float32)
    for dk in range(3):
        conv += pad[:, dk:dk + LW, :] @ conv_w[:, :, dk].T
    conv += conv_b[None, None, :]
    char_feat = conv.max(axis=1).reshape(B, S, CHAR_C)

    # --- word embedding + concat ---
    wemb = word_table[sent_i.reshape(-1)].reshape(B, S, WORD_E)
    feat = np.concatenate([wemb, char_feat], axis=2)  # [B,S,F]

    # --- device: projections + biLSTM + linear -> emissions ---
    gp = _GATE_PERM
    wcat = np.concatenate([np.asarray(w_ih_f, np.float32)[gp],
                           np.asarray(w_ih_r, np.float32)[gp]], axis=0)
    bcat = np.concatenate([np.asarray(b_f, np.float32)[gp],
                           np.asarray(b_r, np.float32)[gp]])
    wT = np.empty((KA, NW), np.float32)
    wT[:F] = wcat.T
    wT[F] = bcat
    wT = wT.astype(f8)
    whhT_f_a = np.ascontiguousarray(
        np.asarray(w_hh_f, np.float32)[gp].T).astype(bf)
    whhT_r_a = np.ascontiguousarray(
        np.asarray(w_hh_r, np.float32)[gp].T).astype(bf)
    linT = np.ascontiguousarray(lin_w.T).astype(bf)

    in_maps = []
    for c in range(NCORES):
        fc = feat[c * BC:(c + 1) * BC]  # [BC,S,F]
        fT = np.empty((KA, R), np.float32)
        fT[:F] = fc.transpose(2, 1, 0).reshape(F, R)
        fT[F] = 1.0
        gs, hs = NW // 8, H4 // 8
        in_maps.append(dict(
            featT=fT.astype(f8),
            wT_s=np.ascontiguousarray(wT[:, c * gs:(c + 1) * gs]),
            whhf_s=np.ascontiguousarray(whhT_f_a[:, c * hs:(c + 1) * hs]),
            whhr_s=np.ascontiguousarray(whhT_r_a[:, c * hs:(c + 1) * hs]),
            linT=linT))
    em_shards = _run_device(in_maps)
    em = np.empty((S, B, NCLS), np.float32)
    for c in range(NCORES):
        em[:, c * BC:(c + 1) * BC, :] = (
            em_shards[c].astype(np.float32).reshape(S, BC, NCLS))
    em += lin_b[None, None, :]

    # --- CRF NLL (host) ---
    tg = tag_i.T  # [S,B]
    mk = mask_b.T.astype(np.float32)
    bidx = np.arange(B)
    em_tag = np.take_along_axis(em, tg[..., None], axis=-1)[..., 0]
    tr = trans[tg[:-1], tg[1:]]
    score = start_t[tg[0]] + em_tag[0] + np.sum(
        mk[1:] * (tr + em_tag[1:]), axis=0)
    last = mk.sum(0).astype(np.int64) - 1
    score = score + end_t[tg[last, bidx]]
    alpha = start_t[None, :] + em[0]
    for t in range(1, S):
        nxt = _logsumexp(
            alpha[:, :, None] + trans[None, :, :] + em[t][:, None, :], axis=1)
        alpha = np.where(mk[t][:, None] > 0, nxt, alpha)
    logZ = _logsumexp(alpha + end_t[None, :], axis=1)
    return np.asarray(-np.sum(score - logZ), np.float32)


# revision 16
# speedup vs baseline: 19.8808x; 1.4236x over previous
import numpy as np

# Persistent XLA compilation cache: run_bass_kernel_spmd re-traces its jit
# closure every call, so without this each call pays a full XLA re-compile.
try:
    import jax
    jax.config.update("jax_compilation_cache_dir", "/tmp/jax_comp_cache")
    jax.config.update("jax_persistent_cache_min_entry_size_bytes", -1)
    jax.config.update("jax_persistent_cache_min_compile_time_secs", 0.0)
except Exception:
    pass

# CNN-biLSTM-CRF forward NLL, data-parallel over batch across 8 NeuronCores.
# Device computes the input projections, the full biLSTM recurrence and the
# emission linear layer; host handles embedding gathers, the tiny char-CNN
# and the CRF scan. Only the [1024,25] emissions per core come back, so the
# axon transfer (the dominant cost) is ~26x smaller than shipping gates.

B, S, LW = 64, 128, 20
CHAR_E, CHAR_C = 30, 30
WORD_E = 300
H, NCLS = 256, 25
F = WORD_E + CHAR_C  # 330
KA = F + 1  # ones row folds the gate bias into the matmul
NCORES = 8
BC = B // NCORES  # 8 examples per core
R = BC * S  # 1024 rows per core, time-major: row = t*BC + e
NW = 8 * H  # 2048 = both directions' 4H gates
H4 = 4 * H

# gate order on device: i, f, o, g  (sigmoid on [0:3H), tanh on [3H:4H))
_GATE_PERM = np.concatenate([
    np.arange(0, H),            # i
    np.arange(H, 2 * H),        # f
    np.arange(3 * H, 4 * H),    # o
    np.arange(2 * H, 3 * H),    # g
])


def _build_nc():
    import concourse.bacc as bacc
    import concourse.mybir as mybir
    from concourse import tile

    f32 = mybir.dt.float32
    bf16 = mybir.dt.bfloat16
    f8 = mybir.dt.float8e4
    AF = mybir.ActivationFunctionType

    nc = bacc.Bacc("TRN2", target_bir_lowering=False, debug=False,
                   num_devices=NCORES)
    # weights arrive sharded along the gate dim; AllGather on device
    fw = nc.dram_tensor("fw", [KA, R + NW // 8], f8, kind="ExternalInput")
    whh_s = nc.dram_tensor("whh_s", [2 * H, H4 // 8], bf16,
                           kind="ExternalInput")
    linT = nc.dram_tensor("linT", [2 * H, NCLS], bf16, kind="ExternalInput")
    em = nc.dram_tensor("em", [R, NCLS], bf16, kind="ExternalOutput")
    # collectives can't touch I/O tensors: bounce in, gather to Shared
    wT_b = nc.dram_tensor("wT_b", [KA, NW // 8], f8)
    whh_b = nc.dram_tensor("whh_b", [2 * H, H4 // 8], bf16)
    wT_g = nc.dram_tensor("wT_g", [NCORES * KA, NW // 8], f8,
                          addr_space="Shared")
    whh_g = nc.dram_tensor("whh_g", [NCORES * 2 * H, H4 // 8], bf16,
                           addr_space="Shared")
    GRP = [list(range(NCORES))]
    BYP = mybir.AluOpType.bypass

    ksz = [128, 128, KA - 256]
    ko = [0, 128, 256]

    with tile.TileContext(nc) as tc:
        with tc.tile_pool(name="wp", bufs=1) as wp, \
             tc.tile_pool(name="gxp", bufs=1) as gxp, \
             tc.tile_pool(name="stp", bufs=1) as stp, \
             tc.tile_pool(name="wk", bufs=3) as wk, \
             tc.tile_pool(name="pm", bufs=2, space="PSUM") as pm, \
             tc.tile_pool(name="pg", bufs=2, space="PSUM") as pg:

            # AllGather the weight shards (each core holds 1/8 of the gates)
            nc.sync.dma_start(wT_b[:, :], fw[:, R:R + NW // 8])
            nc.sync.dma_start(whh_b[:, :], whh_s[:, :])
            nc.gpsimd.collective_compute("AllGather", BYP, GRP,
                                         ins=[wT_b[:, :]], outs=[wT_g[:, :]])
            nc.gpsimd.collective_compute("AllGather", BYP, GRP,
                                         ins=[whh_b[:, :]],
                                         outs=[whh_g[:, :]])

            # moving featT K-tiles and stationary wT K-tiles
            ft, wt = [], []
            for k in range(3):
                t1 = wp.tile([ksz[k], R], f8, name=f"ft{k}", tag=f"ft{k}")
                nc.sync.dma_start(t1[:, :], fw[ko[k]:ko[k] + ksz[k], 0:R])
                ft.append(t1)
                t2 = wp.tile([ksz[k], NW], f8, name=f"wt{k}", tag=f"wt{k}")
                for c in range(NCORES):
                    nc.sync.dma_start(
                        t2[:, c * (NW // 8):(c + 1) * (NW // 8)],
                        wT_g[c * KA + ko[k]:c * KA + ko[k] + ksz[k], :])
                wt.append(t2)
            whh = {}
            for d in range(2):
                t0 = wp.tile([128, H4], bf16, name=f"whh{d}0", tag=f"whh{d}0")
                t1 = wp.tile([128, H4], bf16, name=f"whh{d}1", tag=f"whh{d}1")
                for c in range(NCORES):
                    base = c * 2 * H + d * H
                    nc.sync.dma_start(
                        t0[:, c * (H4 // 8):(c + 1) * (H4 // 8)],
                        whh_g[base:base + 128, :])
                    nc.sync.dma_start(
                        t1[:, c * (H4 // 8):(c + 1) * (H4 // 8)],
                        whh_g[base + 128:base + 256, :])
                whh[d] = (t0, t1)
            lint = []
            for k in range(4):
                t = wp.tile([128, NCLS], bf16, name=f"lin{k}", tag=f"lin{k}")
                nc.sync.dma_start(t[:, :], linT[k * 128:(k + 1) * 128, :])
                lint.append(t)

            # gxT per direction: [128, S, 8, BC] fp32
            #   gxT_d[p, t, j, e] = gate (j*128+p) of dir d at time t, example e
            #   within-dir gate-slice order j: i0 i1 f0 f1 o0 o1 g0 g1
            gxT = [gxp.tile([128, S, 8, BC], f32, name=f"gxT{d}", tag=f"gxT{d}")
                   for d in range(2)]
            for d in range(2):
                for j in range(8):
                    gi = d * 8 + j
                    for rc in range(2):
                        ps = pm.tile([128, S // 2, BC], f32, tag="mm")
                        for k in range(3):
                            nc.tensor.matmul(
                                ps[:, :, :],
                                wt[k][:, gi * 128:(gi + 1) * 128],
                                ft[k][:, rc * 512:(rc + 1) * 512],
                                start=(k == 0), stop=(k == 2))
                        nc.scalar.copy(
                            gxT[d][:, rc * (S // 2):(rc + 1) * (S // 2), j, :],
                            ps[:, :, :])

            # hcat[0,1] = fwd h dims 0:128/128:256, hcat[2,3] = rev; col=t*8+e
            hcat = [stp.tile([128, R], bf16, name=f"hcat{j}", tag=f"hcat{j}")
                    for j in range(4)]
            # c state per dir: [128, 2, BC] (h dims 0:128 | 128:256)
            cst = [stp.tile([128, 2, BC], f32, name=f"c{d}", tag=f"c{d}")
                   for d in range(2)]

            from concourse.bass import ds

            def lstm_cell(d, gsl, first, hw_cols):
                """One LSTM cell update for direction d reading gates from
                gsl ([128,(1,)8,BC] pre-activation) and writing h to
                hcat[2d..2d+1][:, hw_cols]."""
                c_sb = cst[d]
                acts = wk.tile([128, 8, BC], f32, name=f"acts{d}",
                               tag=f"acts{d}")
                nc.scalar.activation(acts[:, 0:6, :], gsl[:, 0:6, :],
                                     AF.Sigmoid)
                nc.scalar.activation(acts[:, 6:8, :], gsl[:, 6:8, :], AF.Tanh)
                if first:
                    nc.vector.tensor_mul(c_sb[:, :, :], acts[:, 0:2, :],
                                         acts[:, 6:8, :])
                else:
                    fc = wk.tile([128, 2, BC], f32, name=f"fc{d}", tag=f"fc{d}")
                    nc.vector.tensor_mul(fc[:, :, :], acts[:, 2:4, :],
                                         c_sb[:, :, :])
                    nc.vector.tensor_mul(c_sb[:, :, :], acts[:, 0:2, :],
                                         acts[:, 6:8, :])
                    nc.vector.tensor_add(c_sb[:, :, :], c_sb[:, :, :],
                                         fc[:, :, :])
                th = wk.tile([128, 2, BC], f32, name=f"th{d}", tag=f"th{d}")
                nc.scalar.activation(th[:, :, :], c_sb[:, :, :], AF.Tanh)
                nc.vector.tensor_mul(hcat[2 * d][:, hw_cols],
                                     acts[:, 4, :], th[:, 0, :])
                nc.vector.tensor_mul(hcat[2 * d + 1][:, hw_cols],
                                     acts[:, 5, :], th[:, 1, :])

            # step 0 (no h feedback): gates come straight from gxT
            lstm_cell(0, gxT[0][:, 0], True, slice(0, BC))
            lstm_cell(1, gxT[1][:, S - 1], True, slice((S - 1) * BC, S * BC))

            # steps 1..S-1 as a hardware loop; sv = step index
            with tc.For_i(1, S, 1) as sv:
                for d in range(2):
                    # fwd: tt=sv prev=sv-1 ; rev: tt=S-1-sv prev=S-sv
                    if d == 0:
                        gcol = sv
                        pcol = sv * BC - BC
                        wcol = sv * BC
                    else:
                        gcol = S - 1 - sv
                        pcol = S * BC - sv * BC
                        wcol = (S - 1) * BC - sv * BC
                    ps = pg.tile([128, 8, BC], f32, name=f"rps{d}",
                                 tag=f"rps{d}")
                    w0, w1 = whh[d]
                    h0 = hcat[2 * d][:, ds(pcol, BC)]
                    h1 = hcat[2 * d + 1][:, ds(pcol, BC)]
                    for j in range(8):
                        nc.tensor.matmul(ps[:, j, :],
                                         w0[:, j * 128:(j + 1) * 128], h0,
                                         start=True, stop=False)
                        nc.tensor.matmul(ps[:, j, :],
                                         w1[:, j * 128:(j + 1) * 128], h1,
                                         start=False, stop=True)
                    g_sb = wk.tile([128, 8, BC], f32, name=f"g{d}",
                                   tag=f"g{d}")
                    nc.vector.tensor_add(g_sb[:, :, :], ps[:, :, :],
                                         gxT[d][:, ds(gcol, 1)])
                    lstm_cell(d, g_sb, False, ds(wcol, BC))

            for m in range(8):
                pe = pm.tile([128, NCLS], f32, tag="mm")
                for k in range(4):
                    nc.tensor.matmul(pe[:, :], hcat[k][:, m * 128:(m + 1) * 128],
                                     lint[k][:, :], start=(k == 0),
                                     stop=(k == 3))
                eo = wk.tile([128, NCLS], bf16, tag="emo")
                nc.any.tensor_copy(eo[:, :], pe[:, :])
                nc.sync.dma_start(em[m * 128:(m + 1) * 128, :], eo[:, :])
    nc.compile()
    return nc


_NC_CACHE = {}
LAST_DEVICE_NS = [0]


def _run_device(in_maps):
    import time
    from concourse.bass_utils import run_bass_kernel_spmd
    if "nc" not in _NC_CACHE:
        _NC_CACHE["nc"] = _build_nc()
    nc = _NC_CACHE["nc"]
    t0 = time.time()
    res = run_bass_kernel_spmd(nc, in_maps, core_ids=list(range(NCORES)))
    LAST_DEVICE_NS[0] = int((time.time() - t0) * 1e9)
    return [r["em"] for r in res.results]


def _logsumexp(x, axis):
    m = np.max(x, axis=axis, keepdims=True)
    return (m + np.log(np.sum(np.exp(x - m), axis=axis,
                              keepdims=True))).squeeze(axis)


def kernel(word_table, char_table, conv_w, conv_b, w_ih_f, w_hh_f, b_f,
           w_ih_r, w_hh_r, b_r, lin_w, lin_b, start_t, end_t, trans,
           sent, word, tag, mask):
    import ml_dtypes
    bf = ml_dtypes.bfloat16
    f8 = ml_dtypes.float8_e4m3
    word_table = np.asarray(word_table, np.float32)
    char_table = np.asarray(char_table, np.float32)
    conv_w = np.asarray(conv_w, np.float32)
    conv_b = np.asarray(conv_b, np.float32)
    lin_w = np.asarray(lin_w, np.float32)
    lin_b = np.asarray(lin_b, np.float32)
    start_t = np.asarray(start_t, np.float32)
    end_t = np.asarray(end_t, np.float32)
    trans = np.asarray(trans, np.float32)
    sent_i = np.asarray(sent).astype(np.int64)
    word_i = np.asarray(word).astype(np.int64)
    tag_i = np.asarray(tag).astype(np.int64)
    mask_b = np.asarray(mask).astype(bool)

    # --- char CNN (host: tiny) ---
    ct = char_table.copy()
    ct[0] = 0.0
    cemb = ct[word_i.reshape(-1)].reshape(B * S, LW, CHAR_E)
    pad = np.zeros((B * S, LW + 2, CHAR_E), np.float32)
    pad[:, 1:LW + 1, :] = cemb
    conv = np.zeros((B * S, LW, CHAR_C), np.float32)
    for dk in range(3):
        conv += pad[:, dk:dk + LW, :] @ conv_w[:, :, dk].T
    conv += conv_b[None, None, :]
    char_feat = conv.max(axis=1).reshape(B, S, CHAR_C)

    # --- word embedding + concat ---
    wemb = word_table[sent_i.reshape(-1)].reshape(B, S, WORD_E)
    feat = np.concatenate([wemb, char_feat], axis=2)  # [B,S,F]

    # --- device: projections + biLSTM + linear -> emissions ---
    gp = _GATE_PERM
    wcat = np.concatenate([np.asarray(w_ih_f, np.float32)[gp],
                           np.asarray(w_ih_r, np.float32)[gp]], axis=0)
    bcat = np.concatenate([np.asarray(b_f, np.float32)[gp],
                           np.asarray(b_r, np.float32)[gp]])
    wT = np.empty((KA, NW), np.float32)
    wT[:F] = wcat.T
    wT[F] = bcat
    wT = wT.astype(f8)
    whhT_f_a = np.ascontiguousarray(
        np.asarray(w_hh_f, np.float32)[gp].T).astype(bf)
    whhT_r_a = np.ascontiguousarray(
        np.asarray(w_hh_r, np.float32)[gp].T).astype(bf)
    linT = np.ascontiguousarray(lin_w.T).astype(bf)

    in_maps = []
    for c in range(NCORES):
        fc = feat[c * BC:(c + 1) * BC]  # [BC,S,F]
        fT = np.empty((KA, R), np.float32)
        fT[:F] = fc.transpose(2, 1, 0).reshape(F, R)
        fT[F] = 1.0
        gs, hs = NW // 8, H4 // 8
        fw = np.concatenate([fT.astype(f8), wT[:, c * gs:(c + 1) * gs]],
                            axis=1)
        whhs = np.concatenate([whhT_f_a[:, c * hs:(c + 1) * hs],
                               whhT_r_a[:, c * hs:(c + 1) * hs]], axis=0)
        in_maps.append(dict(fw=np.ascontiguousarray(fw),
                            whh_s=np.ascontiguousarray(whhs), linT=linT))
    em_shards = _run_device(in_maps)
    em = np.empty((S, B, NCLS), np.float32)
    for c in range(NCORES):
        em[:, c * BC:(c + 1) * BC, :] = (
            em_shards[c].astype(np.float32).reshape(S, BC, NCLS))
    em += lin_b[None, None, :]

    # --- CRF NLL (host) ---
    tg = tag_i.T  # [S,B]
    mk = mask_b.T.astype(np.float32)
    bidx = np.arange(B)
    em_tag = np.take_along_axis(em, tg[..., None], axis=-1)[..., 0]
    tr = trans[tg[:-1], tg[1:]]
    score = start_t[tg[0]] + em_tag[0] + np.sum(
        mk[1:] * (tr + em_tag[1:]), axis=0)
    last = mk.sum(0).astype(np.int64) - 1
    score = score + end_t[tg[last, bidx]]
    alpha = start_t[None, :] + em[0]
    for t in range(1, S):
        nxt = _logsumexp(
            alpha[:, :, None] + trans[None, :, :] + em[t][:, None, :], axis=1)
        alpha = np.where(mk[t][:, None] > 0, nxt, alpha)
    logZ = _logsumexp(alpha + end_t[None, :], axis=1)
    return np.asarray(-np.sum(score - logZ), np.float32)
